# revision 34
# baseline (speedup 1.0000x reference)
"""Trainium2 Bass kernel for GNN multi-head cross-attention message passing.

Math (see reference): per edge e: score[e,h,g] = qh[A[e],h,:] . kh[B[e],g,:]
segment-MEAN over destination A -> softmax over g -> att @ vh -> Wc projection.

Algebraic structure: sums[n,h,g] = qh[n,h,:] . S[n,g,:] with
S = (segment_sum of raw k rows) @ Wk^T, so the [E,H,H] score tensor is never
materialized and k is projected once per node after aggregation.  Nodes are
sharded contiguously across the 8 cores (edge lists sharded by destination),
so no collective is needed; the host gathers per-edge k rows into a
dest-sorted fp8 stream.

v5 design, tuned against the TimelineSim cost model (HW-validated
179327 -> 146003 ns, rel err 1.19e-2 vs 2e-2 budget):

Numerics: fp8 per-tensor noise (~3.6% rms) does NOT average away in matmuls,
so fp8 is used only where damped or residual-corrected:
 - score path (q, Wq, k, Wk, U^T roundtrip, and the Pool-written g-tail of
   the score products) is fp8: its noise is damped by the softmax argument
   |mean| ~ 0.1 to a ~0.4% output contribution
 - V path: v and Wv ship as fp8 VALUE + fp8 RESIDUAL pairs at matched
   power-of-2 scales (vh = v8@W8 + v8@Rw8 + r8@W8, ~0.6% error at DoubleRow
   speed); ov^T and Wc stay bf16; output is fp16

Structure (per core: 49 blocks of 128 destination nodes, processed as 25
block PAIRS through a depth-11 software pipeline):
 - edges are packed TWO-DEEP per destination: slot (row, pair-tile) holds up
   to 2 same-destination edges, so one host-built one-hot column drives an
   fp8 DoubleRow matmul whose moving operand broadcasts over the k-tile pair
   (0-stride dim): half the S1 matmuls and PE time of per-edge tiles at the
   same DMA bytes; per-tile destination WINDOWS keep the moving width ~20
 - q + v + v-residual ship in one fp8 stream (768B/part/block); k-rows +
   one-hots are fused per block; all streams DMA in 2-block chunks (the
   single HWDGE queue costs ~625ns per dma_start)
 - qh/vh/S projections accumulate into ONE [P, 2, 768] PSUM region at a
   common x1024 scale (U^T is copied at x8 so S lands x1024): one merged
   ACT copy per pair instead of three per block
 - score d-reduction: 32 identity matmuls (bf16 g-head) + 2x16 fp8
   DoubleRow ident-pair matmuls (g-tail) accumulating in PSUM; exp reads
   PSUM directly and is LAST in ACT's priority order
 - V-phase g-reduction runs on the PE as TRANSPOSING identity matmuls
   (lhsT = p2 g-slice, rhs = identity) accumulating ov^T[(h,d), n]: the
   g-sum and the output transpose are the same instructions, and the
   bf16 out-projection consumes ov^T directly; fp16 out^T DMAs per pair
   (host un-transposes for free)
 - softmax: den via DVE X-reduce, reciprocal on DVE (bf16), att-normalize
   on Pool before the V products (so no post-normalization)
 - elementwise products split DVE (g-head, bf16, 2x mode) / Pool (g-tail,
   fp8, feeding the DoubleRow d-reduce)
 - PSUM banks exactly 8: uT-pair(1) + qs-pair(3) + sc-pair(1) + ov-pair(2)
   + o-pair(1); pair tiles use write-mode start=True only on each region's
   first matmul group
 - engine busy/core: DMA 107us (edge stream 85us dominates, the serial
   floor of this memory-bound problem), PE 107us, DVE 97us, ACT 95us,
   Pool 91us
"""
import numpy as np
import ml_dtypes

import concourse.bass as bass
import concourse.mybir as mybir
import concourse.tile as tile
from concourse.bass_utils import run_bass_kernel_spmd
from concourse.masks import make_identity

# ---------------------------------------------------------------- constants
NCORES = 8
N_NODES = 50000
EMB = 256
H = 8
D = 32
P = 128

NPC = N_NODES // NCORES          # 6250 nodes per core
NB = (NPC + P - 1) // P          # 49 blocks of 128 nodes per core
NPC_PAD = NB * P                 # 6272

FP = mybir.dt.float32
BF = mybir.dt.bfloat16
F16 = mybir.dt.float16
F8 = mybir.dt.float8e4
DR = mybir.MatmulPerfMode.DoubleRow

NP_BF = ml_dtypes.bfloat16
NP_F8 = ml_dtypes.float8_e4m3fn

# power-of-2 fp8 scaling (exact): stream = true * SCALE
SQ = 8.0        # q, v (+ residual), k edge rows
SW_ = 128.0     # fp8 weight matrices
SPROJ = SQ * SW_   # common scale of the merged qh/vh/S PSUM region

# Pool g-shares of the two 2048-elem product tensors
PG_PROD = 2
PG_P2 = 1


# ------------------------------------------------------- sync-wait splitting
# The staged walrus accepts only ONE sync-wait command per instruction.
# Tile attaches several waits to some instructions.  Post-pass: hoist all but
# one wait of each over-limit instruction onto same-engine Drain carriers
# placed immediately before it (engine streams execute in block order, so
# "all waits hold before the instruction runs" is preserved).
_WS_COUNTER = [0]


def _split_sync_waits(nc, maxw=1):
    for f in nc.m.functions:
        for blk in f.blocks:
            insts = blk.instructions
            out = []
            changed = False
            for ins in insts:
                si = ins.sync_info
                if si is not None and len(si.on_wait) > maxw:
                    waits = list(si.on_wait)
                    k = len(waits) - maxw
                    for i in range(0, k, maxw):
                        _WS_COUNTER[0] += 1
                        d = mybir.InstDrain(
                            name=f"I-wsplit-{_WS_COUNTER[0]}", ins=[], outs=[]
                        )
                        d.engine = ins.engine
                        d.sync_info = mybir.SyncInfo(
                            on_wait=waits[i : i + maxw], on_update=[]
                        )
                        out.append(d)
                    si.on_wait = waits[k:]
                    changed = True
                out.append(ins)
            if changed:
                blk.instructions = out


# ------------------------------------------------------------- device kernel
def build_nc(pairs_per_block, windows, bias_flags, split_waits=True):
    """Build the SPMD Bass module.

    pairs_per_block[b] = 2-deep edge pair-tiles in block b (same across
    cores).  windows[b] = list of (doff, W) per pair-tile (cross-core
    union).  bias_flags = (has_bq, has_bk, has_bv, has_bc).
    """
    SW = [int(sum(w for _, w in wb)) for wb in windows]   # one-hot cols/block
    has_bq, has_bk, has_bv, has_bc = bias_flags

    nc = bass.Bass("TRN2", target_bir_lowering=False, debug=False,
                   num_devices=NCORES)

    # per-core inputs; qvr/ko/out DMA in 2-block pairs (single shared HWDGE
    # queue at ~625ns per dma_start)
    # qvr j-dim: (q8-lo, q8-hi, v8-lo, v8-hi, r8-lo, r8-hi), all *SQ
    qvr_d = nc.dram_tensor("qvr", [P, NB, 6, P], F8, kind="ExternalInput")
    KOW = [int(pairs_per_block[b]) * 2 * EMB + SW[b] for b in range(NB)]
    ko_d = nc.dram_tensor("ko", [P, sum(KOW)], F8, kind="ExternalInput")
    # fp8 weights [ch%128, ch//128, out] * SW_
    Wq8 = nc.dram_tensor("Wq8", [P, 2, EMB], F8, kind="ExternalInput")
    Wk8 = nc.dram_tensor("Wk8", [P, 2, EMB], F8, kind="ExternalInput")
    Wv8 = nc.dram_tensor("Wv8", [P, 2, EMB], F8, kind="ExternalInput")  # perm
    Rv8 = nc.dram_tensor("Rv8", [P, 2, EMB], F8, kind="ExternalInput")  # perm
    # bf16 Wc^T [(h,d)%128, (h,d)//128, c'] true scale
    Wc16 = nc.dram_tensor("Wc16", [P, 2, EMB], BF, kind="ExternalInput")
    invc_d = nc.dram_tensor("invc", [P, NB], FP, kind="ExternalInput")
    if has_bq or has_bv:
        bqv_d = nc.dram_tensor("bqv", [1, 2 * EMB], BF, kind="ExternalInput")
    if has_bk:
        bk_d = nc.dram_tensor("bk", [1, EMB], BF, kind="ExternalInput")
        cnt_d = nc.dram_tensor("cnt", [1, NPC_PAD], BF, kind="ExternalInput")
    if has_bc:
        bc_d = nc.dram_tensor("bc", [1, EMB], BF, kind="ExternalInput")

    outT_d = nc.dram_tensor("outT", [P, NB, 2, P], F16, kind="ExternalOutput")

    ko_off = [0]
    for b in range(NB):
        ko_off.append(ko_off[-1] + KOW[b])
    KOW2MAX = max(KOW[b] + (KOW[b + 1] if b + 1 < NB else 0)
                  for b in range(0, NB, 2))

    with tile.TileContext(nc) as tc:
        with (
            tc.tile_pool(name="const", bufs=1) as cp,
            tc.tile_pool(name="work", bufs=6) as wp,
            tc.tile_pool(name="qvl", bufs=9) as ql,
            tc.tile_pool(name="kep", bufs=4) as kp,
            tc.tile_pool(name="prd", bufs=4) as pr,
            tc.tile_pool(name="ps_u", bufs=1, space="PSUM") as pu,
            tc.tile_pool(name="ps_qs", bufs=1, space="PSUM") as pqs,
            tc.tile_pool(name="ps_sc", bufs=1, space="PSUM") as psc,
            tc.tile_pool(name="ps_ov", bufs=2, space="PSUM") as pov,
            tc.tile_pool(name="ps_o", bufs=1, space="PSUM") as po,
        ):
            # ---------------- constants
            ident = cp.tile([P, P], BF)
            make_identity(nc, ident[:])
            ident82 = cp.tile([P, 2, P], F8)     # identity pair for DR d-red
            nc.scalar.copy(ident82[:, 0, :], ident[:])
            nc.scalar.copy(ident82[:, 1, :], ident[:])
            zf82 = cp.tile([P, 2, P], F8)
            nc.vector.memset(zf82[:], 0.0)
            if has_bq or has_bv or has_bk or has_bc:
                ones1 = cp.tile([1, P], BF)
                nc.vector.memset(ones1[:], 1.0)

            wt = {}
            for nm, t, dt_ in (("Wq", Wq8, F8), ("Wk", Wk8, F8),
                               ("Wv", Wv8, F8), ("Rv", Rv8, F8),
                               ("Wc", Wc16, BF)):
                s = cp.tile([P, 2, EMB], dt_, tag=f"w{nm}")
                nc.sync.dma_start(s[:], t[:])
                wt[nm] = s
            invc_sb = cp.tile([P, NB], FP)
            nc.sync.dma_start(invc_sb[:], invc_d[:])
            if has_bq or has_bv:
                bqv_sb = cp.tile([1, 2 * EMB], BF, tag="bqv")
                nc.sync.dma_start(bqv_sb[:], bqv_d[:])
            if has_bk:
                bk_sb = cp.tile([1, EMB], BF, tag="bk")
                nc.sync.dma_start(bk_sb[:], bk_d[:])
                cnt_sb = cp.tile([1, NPC_PAD], BF)
                nc.sync.dma_start(cnt_sb[:], cnt_d[:])
            if has_bc:
                bc_sb = cp.tile([1, EMB], BF, tag="bc")
                nc.sync.dma_start(bc_sb[:], bc_d[:])

            st = {}

            # ---------------- stages (software pipeline over block PAIRS)
            # Post-S1 stages process a pair of blocks per op: ACT/DVE/Pool
            # per-op init costs are paid once per pair, and PSUM pair-tiles
            # use first-writer-zeroes (start=True only on each bank's first
            # matmul group; later groups accumulate onto the pending-zero
            # bytes, the same HW-proven idiom as the S1 window resets).
            def S0(m):      # SP: fetch pair m (blocks 2m, 2m+1)
                b = 2 * m
                qvr = ql.tile([P, 2, 6, P], F8, tag="qvr")
                hi = min(b + 2, NB)
                nc.sync.dma_start(qvr[:, 0:hi - b, :, :], qvr_d[:, b:hi, :, :])
                ko = kp.tile([P, KOW2MAX], F8, tag="ko")
                w = ko_off[hi] - ko_off[b]
                nc.sync.dma_start(ko[:, 0:w], ko_d[:, ko_off[b]:ko_off[hi]])
                st["qvr", m] = qvr
                st["ko", m] = ko

            def blocks_of(m):
                b0 = 2 * m
                return [b0] if b0 + 1 >= NB else [b0, b0 + 1]

            def S1(m):      # PE: U^T pair accumulation (DR, shared one-hot)
                ko = st.pop(("ko", m))
                blks = blocks_of(m)
                ps_uT = pu.tile([P, 2, 2, P], FP, space="PSUM", tag="uT")
                # full-width zero reset (write-mode), then accumulate
                mms = []
                for j in range(2):
                    for hf in range(2):
                        mms.append(dict(out=ps_uT[:, j, hf, :], lhsT=zf82[:],
                                        rhs=zf82[:], start=True, stop=False,
                                        perf_mode=DR, skip_group_check=True))
                for b in blks:
                    base = ko_off[b] - ko_off[2 * m]
                    T = int(pairs_per_block[b])
                    ke = ko[:, base:base + T * 2 * EMB].rearrange(
                        "p (t two c) -> p t two c", two=2, c=EMB)
                    oh = ko[:, base + T * 2 * EMB:base + T * 2 * EMB + SW[b]]
                    wo = 0
                    for t in range(T):
                        doff, W = windows[b][t]
                        if W > 0:
                            ohb = oh[:, wo:wo + W].unsqueeze(1).to_broadcast(
                                [P, 2, W])
                            for hf in range(2):
                                mms.append(dict(
                                    out=ps_uT[:, b % 2, hf, doff:doff + W],
                                    lhsT=ke[:, t, :, hf * P:(hf + 1) * P],
                                    rhs=ohb, start=False, stop=False,
                                    perf_mode=DR, skip_group_check=True))
                        wo += W
                    mms[-1]["stop"] = b == blks[-1]
                for kw in mms:
                    nc.tensor.matmul(**kw)
                st["ps_uT", m] = ps_uT

            def S1c(m):     # ACT op1: U^T pair -> SBUF fp8 (stays *SQ)
                ps_uT = st.pop(("ps_uT", m))
                uT_sb = wp.tile([P, 2, 2, P], F8, tag="uT_sb")
                nc.scalar.copy(uT_sb[:], ps_uT[:])
                st["uT_sb", m] = uT_sb

            def S2(m):      # PE: q/v/S projections, pair -> one x1024 region
                qvr = st.pop(("qvr", m))
                uT_sb = st.pop(("uT_sb", m))
                ps_qs = pqs.tile([P, 2, 3 * EMB], FP, space="PSUM", tag="qs")
                for j, b in enumerate(blocks_of(m)):
                    reg = ps_qs[:, j, :]
                    nc.tensor.matmul(out=reg[:, 0:EMB],
                                     lhsT=qvr[:, j, 0:2, :], rhs=wt["Wq"][:],
                                     start=True, stop=not has_bq,
                                     perf_mode=DR, skip_group_check=True)
                    if has_bq:
                        nc.tensor.matmul(out=reg[:, 0:EMB], lhsT=ones1[:],
                                         rhs=bqv_sb[:, 0:EMB],
                                         start=False, stop=True,
                                         skip_group_check=True)
                    nc.tensor.matmul(out=reg[:, EMB:2 * EMB],
                                     lhsT=qvr[:, j, 2:4, :], rhs=wt["Wv"][:],
                                     start=True, stop=False,
                                     perf_mode=DR, skip_group_check=True)
                    nc.tensor.matmul(out=reg[:, EMB:2 * EMB],
                                     lhsT=qvr[:, j, 2:4, :], rhs=wt["Rv"][:],
                                     start=False, stop=False,
                                     perf_mode=DR, skip_group_check=True)
                    nc.tensor.matmul(out=reg[:, EMB:2 * EMB],
                                     lhsT=qvr[:, j, 4:6, :], rhs=wt["Wv"][:],
                                     start=False, stop=not has_bv,
                                     perf_mode=DR, skip_group_check=True)
                    if has_bv:
                        nc.tensor.matmul(out=reg[:, EMB:2 * EMB],
                                         lhsT=ones1[:],
                                         rhs=bqv_sb[:, EMB:2 * EMB],
                                         start=False, stop=True,
                                         skip_group_check=True)
                    nc.tensor.matmul(out=reg[:, 2 * EMB:3 * EMB],
                                     lhsT=uT_sb[:, j, :, :], rhs=wt["Wk"][:],
                                     start=True, stop=not has_bk,
                                     perf_mode=DR, skip_group_check=True)
                    if has_bk:
                        nc.tensor.matmul(
                            out=reg[:, 2 * EMB:3 * EMB],
                            lhsT=cnt_sb[:, b * P:(b + 1) * P],
                            rhs=bk_sb[:], start=False, stop=True,
                            skip_group_check=True)
                st["ps_qs", m] = ps_qs

            def S2c(m):     # ACT op2: merged qh/vh/S pair copy
                ps_qs = st.pop(("ps_qs", m))
                qs_sb = ql.tile([P, 2, 3 * EMB], BF, tag="qs_sb")
                nc.scalar.mul(qs_sb[:], ps_qs[:], 1.0 / SPROJ)
                st["qs_sb", m] = qs_sb

            def S3b(m):     # DVE: prod bf16; Pool: fp8 g-tail (per half
                            # -- ISA free-dim patterns are 3D max)
                qs_sb = st[("qs_sb", m)]
                gs = H - PG_PROD
                prod = pr.tile([P, 2, H, gs, D], BF, tag="prod")
                prod8 = pr.tile([P, 2, H, PG_PROD, D], F8, tag="prod8")
                for j in range(2):
                    qh = qs_sb[:, j, 0:EMB].rearrange(
                        "p (h d) -> p h d", h=H)
                    sg = qs_sb[:, j, 2 * EMB:3 * EMB].rearrange(
                        "p (g d) -> p g d", g=H)
                    nc.vector.tensor_tensor(
                        out=prod[:, j],
                        in0=qh.unsqueeze(2).to_broadcast([P, H, gs, D]),
                        in1=sg[:, 0:gs, :].unsqueeze(1).to_broadcast(
                            [P, H, gs, D]),
                        op=mybir.AluOpType.mult)
                    nc.gpsimd.tensor_tensor(
                        out=prod8[:, j],
                        in0=qh.unsqueeze(2).to_broadcast(
                            [P, H, PG_PROD, D]),
                        in1=sg[:, gs:H, :].unsqueeze(1).to_broadcast(
                            [P, H, PG_PROD, D]),
                        op=mybir.AluOpType.mult)
                st["prod", m] = prod
                st["prod8", m] = prod8

            def S3p(m):     # PE: pair d-red (32 bf16 + 2x16 fp8-DR)
                prod = st.pop(("prod", m))
                prod8 = st.pop(("prod8", m))
                gs = H - PG_PROD
                ps_sc = psc.tile([P, 2, H, H], FP, space="PSUM", tag="sc")
                for dd in range(D):
                    nc.tensor.matmul(out=ps_sc[:, :, :, 0:gs], lhsT=ident[:],
                                     rhs=prod[:, :, :, :, dd],
                                     start=(dd == 0), stop=False,
                                     skip_group_check=True)
                for j in range(2):
                    for jj in range(D // 2):
                        nc.tensor.matmul(
                            out=ps_sc[:, j, :, gs:H],
                            lhsT=ident82[:],
                            rhs=prod8[:, j, :, :, 2 * jj:2 * jj + 2
                                      ].rearrange("p h g d -> p d (h g)"),
                            start=(jj == 0),
                            stop=(jj == D // 2 - 1) and j == 1,
                            perf_mode=DR, skip_group_check=True)
                st["ps_sc", m] = ps_sc

            def S3f(m):     # ACT (last): per-block exp from PSUM
                ps_sc = st.pop(("ps_sc", m))
                ex = wp.tile([P, 2, H, H], BF, tag="ex")
                for j, b in enumerate(blocks_of(m)):
                    nc.scalar.activation(
                        out=ex[:, j, :, :], in_=ps_sc[:, j, :, :],
                        func=mybir.ActivationFunctionType.Exp,
                        scale=invc_sb[:, b:b + 1])
                st["ex", m] = ex

            def S4r(m):     # DVE: pair den + rden; Pool: att = ex * rden
                ex = st.pop(("ex", m))
                den = wp.tile([P, 2, H], FP, tag="den")
                nc.vector.tensor_reduce(den[:], ex[:],
                                        axis=mybir.AxisListType.X,
                                        op=mybir.AluOpType.add)
                rden = wp.tile([P, 2, H], BF, tag="rden")
                with nc.allow_low_precision(
                        reason="bf16 softmax normalization, 2^-8 rel err"):
                    nc.vector.reciprocal(rden[:], den[:])
                att = wp.tile([P, 2, H, H], BF, tag="att")
                nc.gpsimd.tensor_tensor(
                    out=att[:], in0=ex[:],
                    in1=rden[:].unsqueeze(3).to_broadcast([P, 2, H, H]),
                    op=mybir.AluOpType.mult)
                st["att", m] = att

            def S4b(m):     # DVE+Pool: p2 = att (x) vh (per half)
                qs_sb = st.pop(("qs_sb", m))
                att = st.pop(("att", m))
                p2 = pr.tile([P, 2, H, D, H], BF, tag="p2")
                gs = H - PG_P2
                for j in range(2):
                    vh = qs_sb[:, j, EMB:2 * EMB].rearrange(
                        "p (d g) -> p d g", d=D)
                    nc.vector.tensor_tensor(
                        out=p2[:, j, :, :, 0:gs],
                        in0=att[:, j, :, 0:gs].unsqueeze(2).to_broadcast(
                            [P, H, D, gs]),
                        in1=vh[:, :, 0:gs].unsqueeze(1).to_broadcast(
                            [P, H, D, gs]),
                        op=mybir.AluOpType.mult)
                    nc.gpsimd.tensor_tensor(
                        out=p2[:, j, :, :, gs:H],
                        in0=att[:, j, :, gs:H].unsqueeze(2).to_broadcast(
                            [P, H, D, PG_P2]),
                        in1=vh[:, :, gs:H].unsqueeze(1).to_broadcast(
                            [P, H, D, PG_P2]),
                        op=mybir.AluOpType.mult)
                st["p2", m] = p2

            def S5a(m):     # PE: pair transposing g-sum -> ov^T in PSUM
                p2 = st.pop(("p2", m))
                ps_ov = pov.tile([P, 2, 2, P], FP, space="PSUM", tag="ov")
                for j, b in enumerate(blocks_of(m)):
                    for hf in range(2):
                        for g in range(H):
                            lhsT = p2[:, j, 4 * hf:4 * hf + 4, :, g
                                      ].rearrange("p h d -> p (h d)")
                            nc.tensor.matmul(
                                out=ps_ov[:, j, hf, :], lhsT=lhsT,
                                rhs=ident[:],
                                start=(g == 0), stop=(g == H - 1),
                                skip_group_check=True)
                st["ps_ov", m] = ps_ov

            def S5b(m):     # ACT op3: ov^T pair -> SBUF bf16
                ps_ov = st.pop(("ps_ov", m))
                ovT = wp.tile([P, 2, 2, P], BF, tag="ovT")
                nc.scalar.copy(ovT[:], ps_ov[:])
                st["ovT", m] = ovT

            def S5c(m):     # PE: pair out^T = Wc^T @ ov^T (bf16)
                ovT = st.pop(("ovT", m))
                ps_o = po.tile([P, 2, 2, P], FP, space="PSUM", tag="o")
                for j, b in enumerate(blocks_of(m)):
                    for cf in range(2):
                        for hh in range(2):
                            nc.tensor.matmul(
                                out=ps_o[:, j, cf, :],
                                lhsT=wt["Wc"][:, hh, cf * P:(cf + 1) * P],
                                rhs=ovT[:, j, hh, :],
                                start=(hh == 0),
                                stop=(hh == 1) and not has_bc,
                                skip_group_check=True)
                        if has_bc:
                            nc.tensor.matmul(
                                out=ps_o[:, j, cf, :],
                                lhsT=bc_sb[:, cf * P:(cf + 1) * P],
                                rhs=ones1[:],
                                start=False, stop=True,
                                skip_group_check=True)
                st["ps_o", m] = ps_o

            def S5d(m):     # ACT op4: fp16 pair out; SP: DMA pair
                ps_o = st.pop(("ps_o", m))
                fo = ql.tile([P, 2, 2, P], F16, tag="fo")
                nc.scalar.copy(fo[:], ps_o[:])
                b = 2 * m
                hi = min(b + 2, NB)
                nc.sync.dma_start(outT_d[:, b:hi, :, :], fo[:, 0:hi - b, :, :])

            # list order = per-engine priority order; exp (S3f) last so its
            # wait on the same-iteration d-reduce doesn't delay the copies
            import os
            _p = os.environ.get("KPERM", "2")
            if _p == "0":
                stages = [(0, S0), (3, S2), (1, S1), (2, S1c), (3, S2c),
                          (4, S3b), (5, S3p), (6, S4r), (7, S4b), (8, S5a),
                          (9, S5b), (9, S5c), (10, S5d), (5, S3f)]
            elif _p == "1":   # copies earlier in priority
                stages = [(0, S0), (2, S1c), (3, S2c), (9, S5b), (3, S2),
                          (1, S1), (4, S3b), (5, S3p), (6, S4r), (7, S4b),
                          (8, S5a), (9, S5c), (10, S5d), (5, S3f)]
            elif _p == "2":   # tail stages high priority
                stages = [(0, S0), (9, S5b), (9, S5c), (10, S5d), (8, S5a),
                          (7, S4b), (6, S4r), (5, S3p), (4, S3b), (3, S2),
                          (3, S2c), (2, S1c), (1, S1), (5, S3f)]
            elif _p == "3":   # DVE/Pool work first
                stages = [(0, S0), (4, S3b), (7, S4b), (6, S4r), (3, S2),
                          (1, S1), (2, S1c), (3, S2c), (5, S3p), (8, S5a),
                          (9, S5b), (9, S5c), (10, S5d), (5, S3f)]
            DEPTH = 11
            import os as _os
            _pf = _os.environ.get("KPF", "1") == "1"
            NP2 = (NB + 1) // 2
            for i in range(NP2 + DEPTH - 1):
                for off, fn in stages:
                    if fn is S0 and _pf:
                        if i == 0:
                            fn(0)
                        if i + 1 < NP2:
                            fn(i + 1)
                        continue
                    mm = i - off
                    if 0 <= mm < NP2:
                        fn(mm)

    if split_waits:
        _split_sync_waits(nc)
    return nc


# --------------------------------------------------------------- host prep
def _prep(q, k, v, edge_index, Wq, bq, Wk, bk, Wv, bv, Wc, bc):
    A = np.asarray(edge_index[0], dtype=np.int64)
    B = np.asarray(edge_index[1], dtype=np.int64)
    order = np.argsort(A, kind="stable")
    A_s = A[order]
    B_s = B[order]

    core_lo = np.searchsorted(A_s, np.arange(NCORES) * NPC, side="left")
    core_hi = np.searchsorted(A_s, (np.arange(NCORES) + 1) * NPC, side="left")

    # --- per-core 2-deep slot assignment (vectorized)
    per_core = []
    npair = np.zeros((NCORES, NB), dtype=np.int64)
    for o in range(NCORES):
        a = A_s[core_lo[o]:core_hi[o]] - o * NPC      # local dest, ascending
        bi = B_s[core_lo[o]:core_hi[o]]
        n = len(a)
        first = np.searchsorted(a, a, side="left")
        rank = np.arange(n) - first
        depth = rank % 2
        cnt_d = np.bincount(a, minlength=NPC_PAD)
        s_d = (cnt_d + 1) // 2
        s_cum = np.cumsum(s_d) - s_d                   # global slot prefix
        blk_start = s_cum[np.arange(NB) * P]           # first slot of block
        slot_in_blk = (s_cum[a] - blk_start[a // P]) + rank // 2
        t = slot_in_blk // P
        p = slot_in_blk % P
        blk = a // P
        np.maximum.at(npair[o], blk, t + 1)
        per_core.append((a, bi, blk, t, p, depth))
    pairs_per_block = np.maximum(1, npair.max(axis=0)).astype(int)

    # --- per-(block,tile) destination windows, union across cores
    TMAX = int(pairs_per_block.max())
    lo = np.full((NB, TMAX), P, dtype=np.int64)
    hi = np.full((NB, TMAX), -1, dtype=np.int64)
    for o in range(NCORES):
        a, bi, blk, t, p, depth = per_core[o]
        dl = a - blk * P
        np.minimum.at(lo, (blk, t), dl)
        np.maximum.at(hi, (blk, t), dl)
    windows = []
    for bidx in range(NB):
        wb = []
        for t in range(int(pairs_per_block[bidx])):
            if hi[bidx, t] < 0:
                wb.append((0, 0))
                continue
            doff = int(lo[bidx, t])
            W = int(hi[bidx, t]) - doff + 1
            W = min((W + 3) // 4 * 4, P - doff)
            wb.append((doff, W))
        windows.append(wb)
    SW = [sum(w for _, w in wb) for wb in windows]

    KOW = [int(pairs_per_block[b]) * 2 * EMB + SW[b] for b in range(NB)]
    ko_off = np.zeros(NB + 1, dtype=np.int64)
    ko_off[1:] = np.cumsum(KOW)
    ke_base = ko_off[:NB]
    oh_base = ko_off[:NB] + pairs_per_block * 2 * EMB
    oh_col = np.zeros((NB, TMAX), dtype=np.int64)
    doffs = np.zeros((NB, TMAX), dtype=np.int64)
    for bidx in range(NB):
        acc = 0
        for t in range(int(pairs_per_block[bidx])):
            oh_col[bidx, t] = acc
            acc += windows[bidx][t][1]
            doffs[bidx, t] = windows[bidx][t][0]

    k8 = (np.asarray(k, np.float32) * SQ).astype(NP_F8)
    kos = []
    for o in range(NCORES):
        a, bi, blk, t, p, depth = per_core[o]
        ko = np.zeros((P, int(ko_off[-1])), dtype=NP_F8)
        cstart = ke_base[blk] + t * 2 * EMB + depth * EMB
        cidx = cstart[:, None] + np.arange(EMB)[None, :]
        ko[p[:, None], cidx] = k8[bi]
        m0 = depth == 0
        cols = (oh_base[blk[m0]] + oh_col[blk[m0], t[m0]]
                + (a[m0] - blk[m0] * P - doffs[blk[m0], t[m0]]))
        ko[p[m0], cols] = 1.0
        kos.append(ko)

    cnt_nodes = np.bincount(A, minlength=N_NODES).astype(np.float32)
    invc_full = 1.0 / np.maximum(cnt_nodes, 1.0)
    invcs, cnts = [], []
    for o in range(NCORES):
        s = np.ones(NPC_PAD, dtype=np.float32)
        s[:NPC] = invc_full[o * NPC:(o + 1) * NPC]
        invcs.append(np.ascontiguousarray(s.reshape(NB, P).T))
        c = np.zeros(NPC_PAD, dtype=np.float32)
        c[:NPC] = cnt_nodes[o * NPC:(o + 1) * NPC]
        cnts.append((c * SPROJ).reshape(1, NPC_PAD).astype(NP_BF))

    # q fp8; v as fp8 value + fp8 residual (both *SQ, exact power-of-2)
    q8 = (np.asarray(q, np.float32) * SQ).astype(NP_F8)
    vs = np.asarray(v, np.float32) * SQ
    v8 = vs.astype(NP_F8)
    r8 = (vs - v8.astype(np.float32)).astype(NP_F8)
    qvrs = []
    for o in range(NCORES):
        qvr = np.zeros((P, NB, 6, P), dtype=NP_F8)
        for j, src in ((0, q8), (2, v8), (4, r8)):
            sT = np.zeros((EMB, NPC_PAD), dtype=NP_F8)
            sT[:, :NPC] = src[o * NPC:(o + 1) * NPC].T
            qvr[:, :, j, :] = sT[0:P].reshape(P, NB, P)
            qvr[:, :, j + 1, :] = sT[P:EMB].reshape(P, NB, P)
        qvrs.append(qvr)

    # Wv column permutation: vh lands as [n, (d, g)]
    WvT = np.ascontiguousarray(np.asarray(Wv, np.float32).T)
    WvT_perm = WvT.reshape(EMB, H, D).transpose(0, 2, 1).reshape(EMB, EMB)
    bv_perm = np.asarray(bv, np.float32).reshape(H, D).T.reshape(-1)

    def pack2(WT):
        # [ch, out] -> [ch%128, ch//128, out]
        a = np.ascontiguousarray(np.asarray(WT, np.float32)).reshape(
            2, P, EMB)
        return np.ascontiguousarray(a.transpose(1, 0, 2))

    Wv_s = WvT_perm * SW_
    Wv8 = Wv_s.astype(NP_F8)
    Rv8 = (Wv_s - Wv8.astype(np.float32)).astype(NP_F8)

    bias_flags = (bool(np.any(np.asarray(bq))), bool(np.any(np.asarray(bk))),
                  bool(np.any(np.asarray(bv))), bool(np.any(np.asarray(bc))))
    has_bq, has_bk, has_bv, has_bc = bias_flags

    com = {
        "Wq8": pack2(np.asarray(Wq, np.float32).T * SW_).astype(NP_F8),
        "Wk8": pack2(np.asarray(Wk, np.float32).T * SW_).astype(NP_F8),
        "Wv8": pack2(Wv8.astype(np.float32)).astype(NP_F8),
        "Rv8": pack2(Rv8.astype(np.float32)).astype(NP_F8),
        "Wc16": pack2(np.asarray(Wc, np.float32).T).astype(NP_BF),
    }
    if has_bq or has_bv:
        bqv = np.concatenate([np.asarray(bq, np.float32),
                              bv_perm]) * SPROJ
        com["bqv"] = bqv.reshape(1, 2 * EMB).astype(NP_BF)
    if has_bk:
        com["bk"] = np.asarray(bk, np.float32).reshape(1, EMB).astype(NP_BF)
    if has_bc:
        com["bc"] = np.asarray(bc, np.float32).reshape(1, EMB).astype(NP_BF)

    in_maps = []
    for o in range(NCORES):
        m = dict(com)
        m["qvr"] = qvrs[o]
        m["ko"] = kos[o]
        m["invc"] = invcs[o]
        if has_bk:
            m["cnt"] = cnts[o]
        in_maps.append(m)
    return pairs_per_block.tolist(), windows, bias_flags, in_maps


_LAST = {}


def kernel(q, k, v, edge_index, Wq, bq, Wk, bk, Wv, bv, Wc, bc, latent=None,
           _want_results=False, _trace=False):
    pairs_per_block, windows, bias_flags, in_maps = _prep(
        q, k, v, edge_index, Wq, bq, Wk, bk, Wv, bv, Wc, bc)
    key = str((pairs_per_block, windows, bias_flags))
    if _LAST.get("key") != key:
        _LAST["nc"] = build_nc(pairs_per_block, windows, bias_flags)
        _LAST["key"] = key
    nc = _LAST["nc"]

    res = run_bass_kernel_spmd(nc, in_maps, core_ids=list(range(NCORES)),
                               trace=_trace)
    out = np.empty((N_NODES, EMB), dtype=np.float32)
    for o in range(NCORES):
        oT = res.results[o]["outT"].astype(np.float32)   # [P, NB, 2, P]
        full = np.empty((EMB, NPC_PAD), dtype=np.float32)
        full[0:P] = oT[:, :, 0, :].reshape(P, NPC_PAD)
        full[P:EMB] = oT[:, :, 1, :].reshape(P, NPC_PAD)
        out[o * NPC:(o + 1) * NPC] = full[:, :NPC].T
    if _want_results:
        return out, res
    return out


# revision 38
# speedup vs baseline: 1.0043x; 1.0043x over previous
"""Trainium2 Bass kernel for GNN multi-head cross-attention message passing.

Math (see reference): per edge e: score[e,h,g] = qh[A[e],h,:] . kh[B[e],g,:]
segment-MEAN over destination A -> softmax over g -> att @ vh -> Wc projection.

Algebraic structure: sums[n,h,g] = qh[n,h,:] . S[n,g,:] with
S = (segment_sum of raw k rows) @ Wk^T, so the [E,H,H] score tensor is never
materialized and k is projected once per node after aggregation.  Nodes are
sharded contiguously across the 8 cores (edge lists sharded by destination),
so no collective is needed; the host gathers per-edge k rows into a
dest-sorted fp8 stream.

v5 design, tuned against the TimelineSim cost model (HW-validated
179327 -> 146003 ns, rel err 1.19e-2 vs 2e-2 budget):

Numerics: fp8 per-tensor noise (~3.6% rms) does NOT average away in matmuls,
so fp8 is used only where damped or residual-corrected:
 - score path (q, Wq, k, Wk, U^T roundtrip, and the Pool-written g-tail of
   the score products) is fp8: its noise is damped by the softmax argument
   |mean| ~ 0.1 to a ~0.4% output contribution
 - V path: v and Wv ship as fp8 VALUE + fp8 RESIDUAL pairs at matched
   power-of-2 scales (vh = v8@W8 + v8@Rw8 + r8@W8, ~0.6% error at DoubleRow
   speed); ov^T and Wc stay bf16; output is fp16

Structure (per core: 49 blocks of 128 destination nodes, processed as 25
block PAIRS through a depth-11 software pipeline):
 - edges are packed TWO-DEEP per destination: slot (row, pair-tile) holds up
   to 2 same-destination edges, so one host-built one-hot column drives an
   fp8 DoubleRow matmul whose moving operand broadcasts over the k-tile pair
   (0-stride dim): half the S1 matmuls and PE time of per-edge tiles at the
   same DMA bytes; per-tile destination WINDOWS keep the moving width ~20
 - q + v + v-residual ship in one fp8 stream (768B/part/block); k-rows +
   one-hots are fused per block; all streams DMA in 2-block chunks (the
   single HWDGE queue costs ~625ns per dma_start)
 - qh/vh/S projections accumulate into ONE [P, 2, 768] PSUM region at a
   common x1024 scale (U^T is copied at x8 so S lands x1024): one merged
   ACT copy per pair instead of three per block
 - score d-reduction: 32 identity matmuls (bf16 g-head) + 2x16 fp8
   DoubleRow ident-pair matmuls (g-tail) accumulating in PSUM; exp reads
   PSUM directly and is LAST in ACT's priority order
 - V-phase g-reduction runs on the PE as TRANSPOSING identity matmuls
   (lhsT = p2 g-slice, rhs = identity) accumulating ov^T[(h,d), n]: the
   g-sum and the output transpose are the same instructions, and the
   bf16 out-projection consumes ov^T directly; fp16 out^T DMAs per pair
   (host un-transposes for free)
 - softmax: den via DVE X-reduce, reciprocal on DVE (bf16), att-normalize
   on Pool before the V products (so no post-normalization)
 - elementwise products split DVE (g-head, bf16, 2x mode) / Pool (g-tail,
   fp8, feeding the DoubleRow d-reduce)
 - PSUM banks exactly 8: uT-pair(1) + qs-pair(3) + sc-pair(1) + ov-pair(2)
   + o-pair(1); pair tiles use write-mode start=True only on each region's
   first matmul group
 - engine busy/core: DMA 107us (edge stream 85us dominates, the serial
   floor of this memory-bound problem), PE 107us, DVE 97us, ACT 95us,
   Pool 91us
"""
import numpy as np
import ml_dtypes

import concourse.bass as bass
import concourse.mybir as mybir
import concourse.tile as tile
from concourse.bass_utils import run_bass_kernel_spmd
from concourse.masks import make_identity

# ---------------------------------------------------------------- constants
NCORES = 8
N_NODES = 50000
EMB = 256
H = 8
D = 32
P = 128

NPC = N_NODES // NCORES          # 6250 nodes per core
NB = (NPC + P - 1) // P          # 49 blocks of 128 nodes per core
NPC_PAD = NB * P                 # 6272

FP = mybir.dt.float32
BF = mybir.dt.bfloat16
F16 = mybir.dt.float16
F8 = mybir.dt.float8e4
DR = mybir.MatmulPerfMode.DoubleRow

NP_BF = ml_dtypes.bfloat16
NP_F8 = ml_dtypes.float8_e4m3fn

# power-of-2 fp8 scaling (exact): stream = true * SCALE
SQ = 8.0        # q, v (+ residual), k edge rows
SW_ = 128.0     # fp8 weight matrices
SPROJ = SQ * SW_   # common scale of the merged qh/vh/S PSUM region

# Pool g-shares of the two 2048-elem product tensors
PG_PROD = 2
PG_P2 = 1


# ------------------------------------------------------- sync-wait splitting
# The staged walrus accepts only ONE sync-wait command per instruction.
# Tile attaches several waits to some instructions.  Post-pass: hoist all but
# one wait of each over-limit instruction onto same-engine Drain carriers
# placed immediately before it (engine streams execute in block order, so
# "all waits hold before the instruction runs" is preserved).
_WS_COUNTER = [0]


def _split_sync_waits(nc, maxw=1):
    for f in nc.m.functions:
        for blk in f.blocks:
            insts = blk.instructions
            out = []
            changed = False
            for ins in insts:
                si = ins.sync_info
                if si is not None and len(si.on_wait) > maxw:
                    waits = list(si.on_wait)
                    k = len(waits) - maxw
                    for i in range(0, k, maxw):
                        _WS_COUNTER[0] += 1
                        d = mybir.InstDrain(
                            name=f"I-wsplit-{_WS_COUNTER[0]}", ins=[], outs=[]
                        )
                        d.engine = ins.engine
                        d.sync_info = mybir.SyncInfo(
                            on_wait=waits[i : i + maxw], on_update=[]
                        )
                        out.append(d)
                    si.on_wait = waits[k:]
                    changed = True
                out.append(ins)
            if changed:
                blk.instructions = out


# ------------------------------------------------------------- device kernel
def build_nc(pairs_per_block, windows, bias_flags, split_waits=True):
    """Build the SPMD Bass module.

    pairs_per_block[b] = 2-deep edge pair-tiles in block b (same across
    cores).  windows[b] = list of (doff, W) per pair-tile (cross-core
    union).  bias_flags = (has_bq, has_bk, has_bv, has_bc).
    """
    SW = [int(sum(w for _, w in wb)) for wb in windows]   # one-hot cols/block
    has_bq, has_bk, has_bv, has_bc = bias_flags

    nc = bass.Bass("TRN2", target_bir_lowering=False, debug=False,
                   num_devices=NCORES)

    # per-core inputs; qvr/ko/out DMA in 2-block pairs (single shared HWDGE
    # queue at ~625ns per dma_start)
    # qvr j-dim: (q8-lo, q8-hi, v8-lo, v8-hi, r8-lo, r8-hi), all *SQ
    qvr_d = nc.dram_tensor("qvr", [P, NB, 6, P], F8, kind="ExternalInput")
    KOW = [int(pairs_per_block[b]) * 2 * EMB + SW[b] for b in range(NB)]
    ko_d = nc.dram_tensor("ko", [P, sum(KOW)], F8, kind="ExternalInput")
    # fp8 weights [ch%128, ch//128, out] * SW_
    Wq8 = nc.dram_tensor("Wq8", [P, 2, EMB], F8, kind="ExternalInput")
    Wk8 = nc.dram_tensor("Wk8", [P, 2, EMB], F8, kind="ExternalInput")
    Wv8 = nc.dram_tensor("Wv8", [P, 2, EMB], F8, kind="ExternalInput")  # perm
    Rv8 = nc.dram_tensor("Rv8", [P, 2, EMB], F8, kind="ExternalInput")  # perm
    # bf16 Wc^T [(h,d)%128, (h,d)//128, c'] true scale
    Wc16 = nc.dram_tensor("Wc16", [P, 2, EMB], BF, kind="ExternalInput")
    invc_d = nc.dram_tensor("invc", [P, NB], FP, kind="ExternalInput")
    if has_bq or has_bv:
        bqv_d = nc.dram_tensor("bqv", [1, 2 * EMB], BF, kind="ExternalInput")
    if has_bk:
        bk_d = nc.dram_tensor("bk", [1, EMB], BF, kind="ExternalInput")
        cnt_d = nc.dram_tensor("cnt", [1, NPC_PAD], BF, kind="ExternalInput")
    if has_bc:
        bc_d = nc.dram_tensor("bc", [1, EMB], BF, kind="ExternalInput")

    outT_d = nc.dram_tensor("outT", [P, NB, 2, P], F16, kind="ExternalOutput")

    ko_off = [0]
    for b in range(NB):
        ko_off.append(ko_off[-1] + KOW[b])
    KOW2MAX = max(KOW[b] + (KOW[b + 1] if b + 1 < NB else 0)
                  for b in range(0, NB, 2))

    with tile.TileContext(nc) as tc:
        with (
            tc.tile_pool(name="const", bufs=1) as cp,
            tc.tile_pool(name="work", bufs=6) as wp,
            tc.tile_pool(name="qvl", bufs=9) as ql,
            tc.tile_pool(name="kep", bufs=4) as kp,
            tc.tile_pool(name="prd", bufs=4) as pr,
            tc.tile_pool(name="ps_u", bufs=1, space="PSUM") as pu,
            tc.tile_pool(name="ps_qs", bufs=1, space="PSUM") as pqs,
            tc.tile_pool(name="ps_sc", bufs=1, space="PSUM") as psc,
            tc.tile_pool(name="ps_ov", bufs=2, space="PSUM") as pov,
            tc.tile_pool(name="ps_o", bufs=1, space="PSUM") as po,
        ):
            # ---------------- constants
            ident = cp.tile([P, P], BF)
            make_identity(nc, ident[:])
            ident82 = cp.tile([P, 2, P], F8)     # identity pair for DR d-red
            nc.scalar.copy(ident82[:, 0, :], ident[:])
            nc.scalar.copy(ident82[:, 1, :], ident[:])
            zf82 = cp.tile([P, 2, P], F8)
            nc.vector.memset(zf82[:], 0.0)
            if has_bq or has_bv or has_bk or has_bc:
                ones1 = cp.tile([1, P], BF)
                nc.vector.memset(ones1[:], 1.0)

            wt = {}
            for nm, t, dt_ in (("Wq", Wq8, F8), ("Wk", Wk8, F8),
                               ("Wv", Wv8, F8), ("Rv", Rv8, F8),
                               ("Wc", Wc16, BF)):
                s = cp.tile([P, 2, EMB], dt_, tag=f"w{nm}")
                nc.sync.dma_start(s[:], t[:])
                wt[nm] = s
            invc_sb = cp.tile([P, NB], FP)
            nc.sync.dma_start(invc_sb[:], invc_d[:])
            if has_bq or has_bv:
                bqv_sb = cp.tile([1, 2 * EMB], BF, tag="bqv")
                nc.sync.dma_start(bqv_sb[:], bqv_d[:])
            if has_bk:
                bk_sb = cp.tile([1, EMB], BF, tag="bk")
                nc.sync.dma_start(bk_sb[:], bk_d[:])
                cnt_sb = cp.tile([1, NPC_PAD], BF)
                nc.sync.dma_start(cnt_sb[:], cnt_d[:])
            if has_bc:
                bc_sb = cp.tile([1, EMB], BF, tag="bc")
                nc.sync.dma_start(bc_sb[:], bc_d[:])

            st = {}

            # ---------------- stages (software pipeline over block PAIRS)
            # Post-S1 stages process a pair of blocks per op: ACT/DVE/Pool
            # per-op init costs are paid once per pair, and PSUM pair-tiles
            # use first-writer-zeroes (start=True only on each bank's first
            # matmul group; later groups accumulate onto the pending-zero
            # bytes, the same HW-proven idiom as the S1 window resets).
            def S0(m):      # SP: fetch pair m (blocks 2m, 2m+1)
                b = 2 * m
                qvr = ql.tile([P, 2, 6, P], F8, tag="qvr")
                hi = min(b + 2, NB)
                nc.sync.dma_start(qvr[:, 0:hi - b, :, :], qvr_d[:, b:hi, :, :])
                ko = kp.tile([P, KOW2MAX], F8, tag="ko")
                import os as _o
                if _o.environ.get("KSPLIT", "0") == "1":
                    mid = ko_off[min(b + 1, hi)]
                    nc.sync.dma_start(ko[:, 0:mid - ko_off[b]],
                                      ko_d[:, ko_off[b]:mid])
                    if mid < ko_off[hi]:
                        nc.sync.dma_start(
                            ko[:, mid - ko_off[b]:ko_off[hi] - ko_off[b]],
                            ko_d[:, mid:ko_off[hi]])
                else:
                    w = ko_off[hi] - ko_off[b]
                    nc.sync.dma_start(ko[:, 0:w],
                                      ko_d[:, ko_off[b]:ko_off[hi]])
                st["qvr", m] = qvr
                st["ko", m] = ko

            def blocks_of(m):
                b0 = 2 * m
                return [b0] if b0 + 1 >= NB else [b0, b0 + 1]

            def S1(m):      # PE: U^T pair accumulation (DR, shared one-hot)
                ko = st.pop(("ko", m))
                blks = blocks_of(m)
                ps_uT = pu.tile([P, 2, 2, P], FP, space="PSUM", tag="uT")
                # full-width zero reset (write-mode), then accumulate
                mms = []
                for j in range(2):
                    for hf in range(2):
                        mms.append(dict(out=ps_uT[:, j, hf, :], lhsT=zf82[:],
                                        rhs=zf82[:], start=True, stop=False,
                                        perf_mode=DR, skip_group_check=True))
                for b in blks:
                    base = ko_off[b] - ko_off[2 * m]
                    T = int(pairs_per_block[b])
                    ke = ko[:, base:base + T * 2 * EMB].rearrange(
                        "p (t two c) -> p t two c", two=2, c=EMB)
                    oh = ko[:, base + T * 2 * EMB:base + T * 2 * EMB + SW[b]]
                    wo = 0
                    for t in range(T):
                        doff, W = windows[b][t]
                        if W > 0:
                            ohb = oh[:, wo:wo + W].unsqueeze(1).to_broadcast(
                                [P, 2, W])
                            for hf in range(2):
                                mms.append(dict(
                                    out=ps_uT[:, b % 2, hf, doff:doff + W],
                                    lhsT=ke[:, t, :, hf * P:(hf + 1) * P],
                                    rhs=ohb, start=False, stop=False,
                                    perf_mode=DR, skip_group_check=True))
                        wo += W
                    mms[-1]["stop"] = b == blks[-1]
                for kw in mms:
                    nc.tensor.matmul(**kw)
                st["ps_uT", m] = ps_uT

            def S1c(m):     # ACT op1: U^T pair -> SBUF fp8 (stays *SQ)
                ps_uT = st.pop(("ps_uT", m))
                uT_sb = wp.tile([P, 2, 2, P], F8, tag="uT_sb")
                nc.scalar.copy(uT_sb[:], ps_uT[:])
                st["uT_sb", m] = uT_sb

            def S2(m):      # PE: q/v/S projections, pair -> one x1024 region
                qvr = st.pop(("qvr", m))
                uT_sb = st.pop(("uT_sb", m))
                ps_qs = pqs.tile([P, 2, 3 * EMB], FP, space="PSUM", tag="qs")
                for j, b in enumerate(blocks_of(m)):
                    reg = ps_qs[:, j, :]
                    nc.tensor.matmul(out=reg[:, 0:EMB],
                                     lhsT=qvr[:, j, 0:2, :], rhs=wt["Wq"][:],
                                     start=True, stop=not has_bq,
                                     perf_mode=DR, skip_group_check=True)
                    if has_bq:
                        nc.tensor.matmul(out=reg[:, 0:EMB], lhsT=ones1[:],
                                         rhs=bqv_sb[:, 0:EMB],
                                         start=False, stop=True,
                                         skip_group_check=True)
                    nc.tensor.matmul(out=reg[:, EMB:2 * EMB],
                                     lhsT=qvr[:, j, 2:4, :], rhs=wt["Wv"][:],
                                     start=True, stop=False,
                                     perf_mode=DR, skip_group_check=True)
                    nc.tensor.matmul(out=reg[:, EMB:2 * EMB],
                                     lhsT=qvr[:, j, 2:4, :], rhs=wt["Rv"][:],
                                     start=False, stop=False,
                                     perf_mode=DR, skip_group_check=True)
                    nc.tensor.matmul(out=reg[:, EMB:2 * EMB],
                                     lhsT=qvr[:, j, 4:6, :], rhs=wt["Wv"][:],
                                     start=False, stop=not has_bv,
                                     perf_mode=DR, skip_group_check=True)
                    if has_bv:
                        nc.tensor.matmul(out=reg[:, EMB:2 * EMB],
                                         lhsT=ones1[:],
                                         rhs=bqv_sb[:, EMB:2 * EMB],
                                         start=False, stop=True,
                                         skip_group_check=True)
                    nc.tensor.matmul(out=reg[:, 2 * EMB:3 * EMB],
                                     lhsT=uT_sb[:, j, :, :], rhs=wt["Wk"][:],
                                     start=True, stop=not has_bk,
                                     perf_mode=DR, skip_group_check=True)
                    if has_bk:
                        nc.tensor.matmul(
                            out=reg[:, 2 * EMB:3 * EMB],
                            lhsT=cnt_sb[:, b * P:(b + 1) * P],
                            rhs=bk_sb[:], start=False, stop=True,
                            skip_group_check=True)
                st["ps_qs", m] = ps_qs

            def S2c(m):     # ACT op2: merged qh/vh/S pair copy
                ps_qs = st.pop(("ps_qs", m))
                qs_sb = ql.tile([P, 2, 3 * EMB], BF, tag="qs_sb")
                nc.scalar.mul(qs_sb[:], ps_qs[:], 1.0 / SPROJ)
                st["qs_sb", m] = qs_sb

            def S3b(m):     # DVE: prod bf16; Pool: fp8 g-tail (per half
                            # -- ISA free-dim patterns are 3D max)
                qs_sb = st[("qs_sb", m)]
                gs = H - PG_PROD
                prod = pr.tile([P, 2, H, gs, D], BF, tag="prod")
                prod8 = pr.tile([P, 2, H, PG_PROD, D], F8, tag="prod8")
                for j in range(2):
                    qh = qs_sb[:, j, 0:EMB].rearrange(
                        "p (h d) -> p h d", h=H)
                    sg = qs_sb[:, j, 2 * EMB:3 * EMB].rearrange(
                        "p (g d) -> p g d", g=H)
                    nc.vector.tensor_tensor(
                        out=prod[:, j],
                        in0=qh.unsqueeze(2).to_broadcast([P, H, gs, D]),
                        in1=sg[:, 0:gs, :].unsqueeze(1).to_broadcast(
                            [P, H, gs, D]),
                        op=mybir.AluOpType.mult)
                    nc.gpsimd.tensor_tensor(
                        out=prod8[:, j],
                        in0=qh.unsqueeze(2).to_broadcast(
                            [P, H, PG_PROD, D]),
                        in1=sg[:, gs:H, :].unsqueeze(1).to_broadcast(
                            [P, H, PG_PROD, D]),
                        op=mybir.AluOpType.mult)
                st["prod", m] = prod
                st["prod8", m] = prod8

            def S3p(m):     # PE: pair d-red (32 bf16 + 2x16 fp8-DR)
                prod = st.pop(("prod", m))
                prod8 = st.pop(("prod8", m))
                gs = H - PG_PROD
                ps_sc = psc.tile([P, 2, H, H], FP, space="PSUM", tag="sc")
                for dd in range(D):
                    nc.tensor.matmul(out=ps_sc[:, :, :, 0:gs], lhsT=ident[:],
                                     rhs=prod[:, :, :, :, dd],
                                     start=(dd == 0), stop=False,
                                     skip_group_check=True)
                for j in range(2):
                    for jj in range(D // 2):
                        nc.tensor.matmul(
                            out=ps_sc[:, j, :, gs:H],
                            lhsT=ident82[:],
                            rhs=prod8[:, j, :, :, 2 * jj:2 * jj + 2
                                      ].rearrange("p h g d -> p d (h g)"),
                            start=(jj == 0),
                            stop=(jj == D // 2 - 1) and j == 1,
                            perf_mode=DR, skip_group_check=True)
                st["ps_sc", m] = ps_sc

            def S3f(m):     # ACT (last): per-block exp from PSUM
                ps_sc = st.pop(("ps_sc", m))
                ex = wp.tile([P, 2, H, H], BF, tag="ex")
                for j, b in enumerate(blocks_of(m)):
                    nc.scalar.activation(
                        out=ex[:, j, :, :], in_=ps_sc[:, j, :, :],
                        func=mybir.ActivationFunctionType.Exp,
                        scale=invc_sb[:, b:b + 1])
                st["ex", m] = ex

            def S4r(m):     # DVE: pair den + rden; Pool: att = ex * rden
                ex = st.pop(("ex", m))
                den = wp.tile([P, 2, H], FP, tag="den")
                nc.vector.tensor_reduce(den[:], ex[:],
                                        axis=mybir.AxisListType.X,
                                        op=mybir.AluOpType.add)
                rden = wp.tile([P, 2, H], BF, tag="rden")
                with nc.allow_low_precision(
                        reason="bf16 softmax normalization, 2^-8 rel err"):
                    nc.vector.reciprocal(rden[:], den[:])
                att = wp.tile([P, 2, H, H], BF, tag="att")
                nc.gpsimd.tensor_tensor(
                    out=att[:], in0=ex[:],
                    in1=rden[:].unsqueeze(3).to_broadcast([P, 2, H, H]),
                    op=mybir.AluOpType.mult)
                st["att", m] = att

            def S4b(m):     # DVE+Pool: p2 = att (x) vh (per half)
                qs_sb = st.pop(("qs_sb", m))
                att = st.pop(("att", m))
                p2 = pr.tile([P, 2, H, D, H], BF, tag="p2")
                gs = H - PG_P2
                for j in range(2):
                    vh = qs_sb[:, j, EMB:2 * EMB].rearrange(
                        "p (d g) -> p d g", d=D)
                    nc.vector.tensor_tensor(
                        out=p2[:, j, :, :, 0:gs],
                        in0=att[:, j, :, 0:gs].unsqueeze(2).to_broadcast(
                            [P, H, D, gs]),
                        in1=vh[:, :, 0:gs].unsqueeze(1).to_broadcast(
                            [P, H, D, gs]),
                        op=mybir.AluOpType.mult)
                    nc.gpsimd.tensor_tensor(
                        out=p2[:, j, :, :, gs:H],
                        in0=att[:, j, :, gs:H].unsqueeze(2).to_broadcast(
                            [P, H, D, PG_P2]),
                        in1=vh[:, :, gs:H].unsqueeze(1).to_broadcast(
                            [P, H, D, PG_P2]),
                        op=mybir.AluOpType.mult)
                st["p2", m] = p2

            def S5a(m):     # PE: pair transposing g-sum -> ov^T in PSUM
                p2 = st.pop(("p2", m))
                ps_ov = pov.tile([P, 2, 2, P], FP, space="PSUM", tag="ov")
                for j, b in enumerate(blocks_of(m)):
                    for hf in range(2):
                        for g in range(H):
                            lhsT = p2[:, j, 4 * hf:4 * hf + 4, :, g
                                      ].rearrange("p h d -> p (h d)")
                            nc.tensor.matmul(
                                out=ps_ov[:, j, hf, :], lhsT=lhsT,
                                rhs=ident[:],
                                start=(g == 0), stop=(g == H - 1),
                                skip_group_check=True)
                st["ps_ov", m] = ps_ov

            def S5b(m):     # ACT op3: ov^T pair -> SBUF bf16
                ps_ov = st.pop(("ps_ov", m))
                ovT = wp.tile([P, 2, 2, P], BF, tag="ovT")
                nc.scalar.copy(ovT[:], ps_ov[:])
                st["ovT", m] = ovT

            def S5c(m):     # PE: pair out^T = Wc^T @ ov^T (bf16)
                ovT = st.pop(("ovT", m))
                ps_o = po.tile([P, 2, 2, P], FP, space="PSUM", tag="o")
                for j, b in enumerate(blocks_of(m)):
                    for cf in range(2):
                        for hh in range(2):
                            nc.tensor.matmul(
                                out=ps_o[:, j, cf, :],
                                lhsT=wt["Wc"][:, hh, cf * P:(cf + 1) * P],
                                rhs=ovT[:, j, hh, :],
                                start=(hh == 0),
                                stop=(hh == 1) and not has_bc,
                                skip_group_check=True)
                        if has_bc:
                            nc.tensor.matmul(
                                out=ps_o[:, j, cf, :],
                                lhsT=bc_sb[:, cf * P:(cf + 1) * P],
                                rhs=ones1[:],
                                start=False, stop=True,
                                skip_group_check=True)
                st["ps_o", m] = ps_o

            def S5d(m):     # ACT op4: fp16 pair out; SP: DMA pair
                ps_o = st.pop(("ps_o", m))
                fo = ql.tile([P, 2, 2, P], F16, tag="fo")
                nc.scalar.copy(fo[:], ps_o[:])
                b = 2 * m
                hi = min(b + 2, NB)
                nc.sync.dma_start(outT_d[:, b:hi, :, :], fo[:, 0:hi - b, :, :])

            # list order = per-engine priority order; exp (S3f) last so its
            # wait on the same-iteration d-reduce doesn't delay the copies
            import os
            _p = os.environ.get("KPERM", "7")
            if _p == "0":
                stages = [(0, S0), (3, S2), (1, S1), (2, S1c), (3, S2c),
                          (4, S3b), (5, S3p), (6, S4r), (7, S4b), (8, S5a),
                          (9, S5b), (9, S5c), (10, S5d), (5, S3f)]
            elif _p == "1":   # copies earlier in priority
                stages = [(0, S0), (2, S1c), (3, S2c), (9, S5b), (3, S2),
                          (1, S1), (4, S3b), (5, S3p), (6, S4r), (7, S4b),
                          (8, S5a), (9, S5c), (10, S5d), (5, S3f)]
            elif _p == "2":   # tail stages high priority
                stages = [(0, S0), (9, S5b), (9, S5c), (10, S5d), (8, S5a),
                          (7, S4b), (6, S4r), (5, S3p), (4, S3b), (3, S2),
                          (3, S2c), (2, S1c), (1, S1), (5, S3f)]
            elif _p == "4":   # tail-first but copies before compute
                stages = [(0, S0), (10, S5d), (9, S5b), (2, S1c), (3, S2c),
                          (9, S5c), (8, S5a), (7, S4b), (6, S4r), (5, S3p),
                          (4, S3b), (3, S2), (1, S1), (5, S3f)]
            elif _p == "5":   # S0 after tail stages
                stages = [(10, S5d), (9, S5b), (9, S5c), (0, S0), (8, S5a),
                          (7, S4b), (6, S4r), (5, S3p), (4, S3b), (3, S2),
                          (3, S2c), (2, S1c), (1, S1), (5, S3f)]
            elif _p == "7":
                stages = [(0, S0), (10, S5d), (9, S5b), (9, S5c), (8, S5a),
                          (7, S4b), (6, S4r), (5, S3p), (5, S3f), (4, S3b),
                          (3, S2), (3, S2c), (2, S1c), (1, S1)]
            elif _p == "8":
                stages = [(0, S0), (5, S3p), (5, S3f), (9, S5b), (9, S5c),
                          (10, S5d), (8, S5a), (7, S4b), (6, S4r), (4, S3b),
                          (3, S2), (3, S2c), (2, S1c), (1, S1)]
            elif _p == "3":   # DVE/Pool work first
                stages = [(0, S0), (4, S3b), (7, S4b), (6, S4r), (3, S2),
                          (1, S1), (2, S1c), (3, S2c), (5, S3p), (8, S5a),
                          (9, S5b), (9, S5c), (10, S5d), (5, S3f)]
            DEPTH = 11
            import os as _os
            _pf = _os.environ.get("KPF", "1") == "1"
            NP2 = (NB + 1) // 2
            for i in range(NP2 + DEPTH - 1):
                for off, fn in stages:
                    if fn is S0 and _pf:
                        if i == 0:
                            fn(0)
                        if i + 1 < NP2:
                            fn(i + 1)
                        continue
                    mm = i - off
                    if 0 <= mm < NP2:
                        fn(mm)

    if split_waits:
        _split_sync_waits(nc)
    return nc


# --------------------------------------------------------------- host prep
def _prep(q, k, v, edge_index, Wq, bq, Wk, bk, Wv, bv, Wc, bc):
    A = np.asarray(edge_index[0], dtype=np.int64)
    B = np.asarray(edge_index[1], dtype=np.int64)
    order = np.argsort(A, kind="stable")
    A_s = A[order]
    B_s = B[order]

    core_lo = np.searchsorted(A_s, np.arange(NCORES) * NPC, side="left")
    core_hi = np.searchsorted(A_s, (np.arange(NCORES) + 1) * NPC, side="left")

    # --- per-core 2-deep slot assignment (vectorized)
    per_core = []
    npair = np.zeros((NCORES, NB), dtype=np.int64)
    for o in range(NCORES):
        a = A_s[core_lo[o]:core_hi[o]] - o * NPC      # local dest, ascending
        bi = B_s[core_lo[o]:core_hi[o]]
        n = len(a)
        first = np.searchsorted(a, a, side="left")
        rank = np.arange(n) - first
        depth = rank % 2
        cnt_d = np.bincount(a, minlength=NPC_PAD)
        s_d = (cnt_d + 1) // 2
        s_cum = np.cumsum(s_d) - s_d                   # global slot prefix
        blk_start = s_cum[np.arange(NB) * P]           # first slot of block
        slot_in_blk = (s_cum[a] - blk_start[a // P]) + rank // 2
        t = slot_in_blk // P
        p = slot_in_blk % P
        blk = a // P
        np.maximum.at(npair[o], blk, t + 1)
        per_core.append((a, bi, blk, t, p, depth))
    pairs_per_block = np.maximum(1, npair.max(axis=0)).astype(int)

    # --- per-(block,tile) destination windows, union across cores
    TMAX = int(pairs_per_block.max())
    lo = np.full((NB, TMAX), P, dtype=np.int64)
    hi = np.full((NB, TMAX), -1, dtype=np.int64)
    for o in range(NCORES):
        a, bi, blk, t, p, depth = per_core[o]
        dl = a - blk * P
        np.minimum.at(lo, (blk, t), dl)
        np.maximum.at(hi, (blk, t), dl)
    windows = []
    for bidx in range(NB):
        wb = []
        for t in range(int(pairs_per_block[bidx])):
            if hi[bidx, t] < 0:
                wb.append((0, 0))
                continue
            doff = int(lo[bidx, t])
            W = int(hi[bidx, t]) - doff + 1
            W = min((W + 3) // 4 * 4, P - doff)
            wb.append((doff, W))
        windows.append(wb)
    SW = [sum(w for _, w in wb) for wb in windows]

    KOW = [int(pairs_per_block[b]) * 2 * EMB + SW[b] for b in range(NB)]
    ko_off = np.zeros(NB + 1, dtype=np.int64)
    ko_off[1:] = np.cumsum(KOW)
    ke_base = ko_off[:NB]
    oh_base = ko_off[:NB] + pairs_per_block * 2 * EMB
    oh_col = np.zeros((NB, TMAX), dtype=np.int64)
    doffs = np.zeros((NB, TMAX), dtype=np.int64)
    for bidx in range(NB):
        acc = 0
        for t in range(int(pairs_per_block[bidx])):
            oh_col[bidx, t] = acc
            acc += windows[bidx][t][1]
            doffs[bidx, t] = windows[bidx][t][0]

    k8 = (np.asarray(k, np.float32) * SQ).astype(NP_F8)
    kos = []
    for o in range(NCORES):
        a, bi, blk, t, p, depth = per_core[o]
        ko = np.zeros((P, int(ko_off[-1])), dtype=NP_F8)
        cstart = ke_base[blk] + t * 2 * EMB + depth * EMB
        cidx = cstart[:, None] + np.arange(EMB)[None, :]
        ko[p[:, None], cidx] = k8[bi]
        m0 = depth == 0
        cols = (oh_base[blk[m0]] + oh_col[blk[m0], t[m0]]
                + (a[m0] - blk[m0] * P - doffs[blk[m0], t[m0]]))
        ko[p[m0], cols] = 1.0
        kos.append(ko)

    cnt_nodes = np.bincount(A, minlength=N_NODES).astype(np.float32)
    invc_full = 1.0 / np.maximum(cnt_nodes, 1.0)
    invcs, cnts = [], []
    for o in range(NCORES):
        s = np.ones(NPC_PAD, dtype=np.float32)
        s[:NPC] = invc_full[o * NPC:(o + 1) * NPC]
        invcs.append(np.ascontiguousarray(s.reshape(NB, P).T))
        c = np.zeros(NPC_PAD, dtype=np.float32)
        c[:NPC] = cnt_nodes[o * NPC:(o + 1) * NPC]
        cnts.append((c * SPROJ).reshape(1, NPC_PAD).astype(NP_BF))

    # q fp8; v as fp8 value + fp8 residual (both *SQ, exact power-of-2)
    q8 = (np.asarray(q, np.float32) * SQ).astype(NP_F8)
    vs = np.asarray(v, np.float32) * SQ
    v8 = vs.astype(NP_F8)
    r8 = (vs - v8.astype(np.float32)).astype(NP_F8)
    qvrs = []
    for o in range(NCORES):
        qvr = np.zeros((P, NB, 6, P), dtype=NP_F8)
        for j, src in ((0, q8), (2, v8), (4, r8)):
            sT = np.zeros((EMB, NPC_PAD), dtype=NP_F8)
            sT[:, :NPC] = src[o * NPC:(o + 1) * NPC].T
            qvr[:, :, j, :] = sT[0:P].reshape(P, NB, P)
            qvr[:, :, j + 1, :] = sT[P:EMB].reshape(P, NB, P)
        qvrs.append(qvr)

    # Wv column permutation: vh lands as [n, (d, g)]
    WvT = np.ascontiguousarray(np.asarray(Wv, np.float32).T)
    WvT_perm = WvT.reshape(EMB, H, D).transpose(0, 2, 1).reshape(EMB, EMB)
    bv_perm = np.asarray(bv, np.float32).reshape(H, D).T.reshape(-1)

    def pack2(WT):
        # [ch, out] -> [ch%128, ch//128, out]
        a = np.ascontiguousarray(np.asarray(WT, np.float32)).reshape(
            2, P, EMB)
        return np.ascontiguousarray(a.transpose(1, 0, 2))

    Wv_s = WvT_perm * SW_
    Wv8 = Wv_s.astype(NP_F8)
    Rv8 = (Wv_s - Wv8.astype(np.float32)).astype(NP_F8)

    bias_flags = (bool(np.any(np.asarray(bq))), bool(np.any(np.asarray(bk))),
                  bool(np.any(np.asarray(bv))), bool(np.any(np.asarray(bc))))
    has_bq, has_bk, has_bv, has_bc = bias_flags

    com = {
        "Wq8": pack2(np.asarray(Wq, np.float32).T * SW_).astype(NP_F8),
        "Wk8": pack2(np.asarray(Wk, np.float32).T * SW_).astype(NP_F8),
        "Wv8": pack2(Wv8.astype(np.float32)).astype(NP_F8),
        "Rv8": pack2(Rv8.astype(np.float32)).astype(NP_F8),
        "Wc16": pack2(np.asarray(Wc, np.float32).T).astype(NP_BF),
    }
    if has_bq or has_bv:
        bqv = np.concatenate([np.asarray(bq, np.float32),
                              bv_perm]) * SPROJ
        com["bqv"] = bqv.reshape(1, 2 * EMB).astype(NP_BF)
    if has_bk:
        com["bk"] = np.asarray(bk, np.float32).reshape(1, EMB).astype(NP_BF)
    if has_bc:
        com["bc"] = np.asarray(bc, np.float32).reshape(1, EMB).astype(NP_BF)

    in_maps = []
    for o in range(NCORES):
        m = dict(com)
        m["qvr"] = qvrs[o]
        m["ko"] = kos[o]
        m["invc"] = invcs[o]
        if has_bk:
            m["cnt"] = cnts[o]
        in_maps.append(m)
    return pairs_per_block.tolist(), windows, bias_flags, in_maps


_LAST = {}


def kernel(q, k, v, edge_index, Wq, bq, Wk, bk, Wv, bv, Wc, bc, latent=None,
           _want_results=False, _trace=False):
    pairs_per_block, windows, bias_flags, in_maps = _prep(
        q, k, v, edge_index, Wq, bq, Wk, bk, Wv, bv, Wc, bc)
    key = str((pairs_per_block, windows, bias_flags))
    if _LAST.get("key") != key:
        _LAST["nc"] = build_nc(pairs_per_block, windows, bias_flags)
        _LAST["key"] = key
    nc = _LAST["nc"]

    res = run_bass_kernel_spmd(nc, in_maps, core_ids=list(range(NCORES)),
                               trace=_trace)
    out = np.empty((N_NODES, EMB), dtype=np.float32)
    for o in range(NCORES):
        oT = res.results[o]["outT"].astype(np.float32)   # [P, NB, 2, P]
        full = np.empty((EMB, NPC_PAD), dtype=np.float32)
        full[0:P] = oT[:, :, 0, :].reshape(P, NPC_PAD)
        full[P:EMB] = oT[:, :, 1, :].reshape(P, NPC_PAD)
        out[o * NPC:(o + 1) * NPC] = full[:, :NPC].T
    if _want_results:
        return out, res
    return out


# revision 40
# speedup vs baseline: 1.0129x; 1.0086x over previous
"""Trainium2 Bass kernel for GNN multi-head cross-attention message passing.

Math (see reference): per edge e: score[e,h,g] = qh[A[e],h,:] . kh[B[e],g,:]
segment-MEAN over destination A -> softmax over g -> att @ vh -> Wc projection.

Algebraic structure: sums[n,h,g] = qh[n,h,:] . S[n,g,:] with
S = (segment_sum of raw k rows) @ Wk^T, so the [E,H,H] score tensor is never
materialized and k is projected once per node after aggregation.  Nodes are
sharded contiguously across the 8 cores (edge lists sharded by destination),
so no collective is needed; the host gathers per-edge k rows into a
dest-sorted fp8 stream.

v5 design, tuned against the TimelineSim cost model (HW-validated
179327 -> 145378 ns, rel err 1.19e-2 vs 2e-2 budget):

Numerics: fp8 per-tensor noise (~3.6% rms) does NOT average away in matmuls,
so fp8 is used only where damped or residual-corrected:
 - score path (q, Wq, k, Wk, U^T roundtrip, and the Pool-written g-tail of
   the score products) is fp8: its noise is damped by the softmax argument
   |mean| ~ 0.1 to a ~0.4% output contribution
 - V path: v and Wv ship as fp8 VALUE + fp8 RESIDUAL pairs at matched
   power-of-2 scales (vh = v8@W8 + v8@Rw8 + r8@W8, ~0.6% error at DoubleRow
   speed); ov^T and Wc stay bf16; output is fp16

Structure (per core: 49 blocks of 128 destination nodes, processed as 25
block PAIRS through a depth-11 software pipeline):
 - edges are packed TWO-DEEP per destination: slot (row, pair-tile) holds up
   to 2 same-destination edges, so one host-built one-hot column drives an
   fp8 DoubleRow matmul whose moving operand broadcasts over the k-tile pair
   (0-stride dim): half the S1 matmuls and PE time of per-edge tiles at the
   same DMA bytes; per-tile destination WINDOWS keep the moving width ~20
 - q + v + v-residual ship in one fp8 stream (768B/part/block); k-rows +
   one-hots are fused per block; all streams DMA in 2-block chunks (the
   single HWDGE queue costs ~625ns per dma_start)
 - qh/vh/S projections accumulate into ONE [P, 2, 768] PSUM region at a
   common x1024 scale (U^T is copied at x8 so S lands x1024): one merged
   ACT copy per pair instead of three per block
 - score d-reduction: 32 identity matmuls (bf16 g-head) + 2x16 fp8
   DoubleRow ident-pair matmuls (g-tail) accumulating in PSUM; exp reads
   PSUM directly and is LAST in ACT's priority order
 - V-phase g-reduction runs on the PE as TRANSPOSING identity matmuls
   (lhsT = p2 g-slice, rhs = identity) accumulating ov^T[(h,d), n]: the
   g-sum and the output transpose are the same instructions, and the
   bf16 out-projection consumes ov^T directly; fp16 out^T DMAs per pair
   (host un-transposes for free)
 - softmax: den via DVE X-reduce, reciprocal on DVE (bf16), att-normalize
   on Pool before the V products (so no post-normalization)
 - elementwise products split DVE (g-head, bf16, 2x mode) / Pool (g-tail,
   fp8, feeding the DoubleRow d-reduce)
 - PSUM banks exactly 8: uT-pair(1) + qs-pair(3) + sc-pair(1) + ov-pair(2)
   + o-pair(1); pair tiles use write-mode start=True only on each region's
   first matmul group
 - engine busy/core: DMA 107us (edge stream 85us dominates, the serial
   floor of this memory-bound problem), PE 107us, DVE 97us, ACT 95us,
   Pool 91us
"""
import numpy as np
import ml_dtypes

import concourse.bass as bass
import concourse.mybir as mybir
import concourse.tile as tile
from concourse.bass_utils import run_bass_kernel_spmd
from concourse.masks import make_identity

# ---------------------------------------------------------------- constants
NCORES = 8
N_NODES = 50000
EMB = 256
H = 8
D = 32
P = 128

NPC = N_NODES // NCORES          # 6250 nodes per core
NB = (NPC + P - 1) // P          # 49 blocks of 128 nodes per core
NPC_PAD = NB * P                 # 6272

FP = mybir.dt.float32
BF = mybir.dt.bfloat16
F16 = mybir.dt.float16
F8 = mybir.dt.float8e4
DR = mybir.MatmulPerfMode.DoubleRow

NP_BF = ml_dtypes.bfloat16
NP_F8 = ml_dtypes.float8_e4m3fn

# power-of-2 fp8 scaling (exact): stream = true * SCALE
SQ = 8.0        # q, v (+ residual), k edge rows
SW_ = 128.0     # fp8 weight matrices
SPROJ = SQ * SW_   # common scale of the merged qh/vh/S PSUM region

# Pool g-shares of the two 2048-elem product tensors
PG_PROD = 2
PG_P2 = 1


# ------------------------------------------------------- sync-wait splitting
# The staged walrus accepts only ONE sync-wait command per instruction.
# Tile attaches several waits to some instructions.  Post-pass: hoist all but
# one wait of each over-limit instruction onto same-engine Drain carriers
# placed immediately before it (engine streams execute in block order, so
# "all waits hold before the instruction runs" is preserved).
_WS_COUNTER = [0]


def _split_sync_waits(nc, maxw=1):
    for f in nc.m.functions:
        for blk in f.blocks:
            insts = blk.instructions
            out = []
            changed = False
            for ins in insts:
                si = ins.sync_info
                if si is not None and len(si.on_wait) > maxw:
                    waits = list(si.on_wait)
                    k = len(waits) - maxw
                    for i in range(0, k, maxw):
                        _WS_COUNTER[0] += 1
                        d = mybir.InstDrain(
                            name=f"I-wsplit-{_WS_COUNTER[0]}", ins=[], outs=[]
                        )
                        d.engine = ins.engine
                        d.sync_info = mybir.SyncInfo(
                            on_wait=waits[i : i + maxw], on_update=[]
                        )
                        out.append(d)
                    si.on_wait = waits[k:]
                    changed = True
                out.append(ins)
            if changed:
                blk.instructions = out


# ------------------------------------------------------------- device kernel
def build_nc(pairs_per_block, windows, bias_flags, split_waits=True):
    """Build the SPMD Bass module.

    pairs_per_block[b] = 2-deep edge pair-tiles in block b (same across
    cores).  windows[b] = list of (doff, W) per pair-tile (cross-core
    union).  bias_flags = (has_bq, has_bk, has_bv, has_bc).
    """
    SW = [int(sum(w for _, w in wb)) for wb in windows]   # one-hot cols/block
    has_bq, has_bk, has_bv, has_bc = bias_flags

    nc = bass.Bass("TRN2", target_bir_lowering=False, debug=False,
                   num_devices=NCORES)

    # per-core inputs; qvr/ko/out DMA in 2-block pairs (single shared HWDGE
    # queue at ~625ns per dma_start)
    # qvr j-dim: (q8-lo, q8-hi, v8-lo, v8-hi, r8-lo, r8-hi), all *SQ
    qvr_d = nc.dram_tensor("qvr", [P, NB, 6, P], F8, kind="ExternalInput")
    KOW = [int(pairs_per_block[b]) * 2 * EMB + SW[b] for b in range(NB)]
    ko_d = nc.dram_tensor("ko", [P, sum(KOW)], F8, kind="ExternalInput")
    # fp8 weights [ch%128, ch//128, out] * SW_
    Wq8 = nc.dram_tensor("Wq8", [P, 2, EMB], F8, kind="ExternalInput")
    Wk8 = nc.dram_tensor("Wk8", [P, 2, EMB], F8, kind="ExternalInput")
    Wv8 = nc.dram_tensor("Wv8", [P, 2, EMB], F8, kind="ExternalInput")  # perm
    Rv8 = nc.dram_tensor("Rv8", [P, 2, EMB], F8, kind="ExternalInput")  # perm
    # bf16 Wc^T [(h,d)%128, (h,d)//128, c'] true scale
    Wc16 = nc.dram_tensor("Wc16", [P, 2, EMB], BF, kind="ExternalInput")
    invc_d = nc.dram_tensor("invc", [P, NB], FP, kind="ExternalInput")
    if has_bq or has_bv:
        bqv_d = nc.dram_tensor("bqv", [1, 2 * EMB], BF, kind="ExternalInput")
    if has_bk:
        bk_d = nc.dram_tensor("bk", [1, EMB], BF, kind="ExternalInput")
        cnt_d = nc.dram_tensor("cnt", [1, NPC_PAD], BF, kind="ExternalInput")
    if has_bc:
        bc_d = nc.dram_tensor("bc", [1, EMB], BF, kind="ExternalInput")

    outT_d = nc.dram_tensor("outT", [P, NB, 2, P], F16, kind="ExternalOutput")

    ko_off = [0]
    for b in range(NB):
        ko_off.append(ko_off[-1] + KOW[b])
    KOW2MAX = max(KOW[b] + (KOW[b + 1] if b + 1 < NB else 0)
                  for b in range(0, NB, 2))

    with tile.TileContext(nc) as tc:
        with (
            tc.tile_pool(name="const", bufs=1) as cp,
            tc.tile_pool(name="work", bufs=6) as wp,
            tc.tile_pool(name="qvl", bufs=10) as ql,
            tc.tile_pool(name="kep", bufs=5) as kp,
            tc.tile_pool(name="prd", bufs=4) as pr,
            tc.tile_pool(name="ps_u", bufs=1, space="PSUM") as pu,
            tc.tile_pool(name="ps_qs", bufs=1, space="PSUM") as pqs,
            tc.tile_pool(name="ps_sc", bufs=1, space="PSUM") as psc,
            tc.tile_pool(name="ps_ov", bufs=2, space="PSUM") as pov,
            tc.tile_pool(name="ps_o", bufs=1, space="PSUM") as po,
        ):
            # ---------------- constants
            ident = cp.tile([P, P], BF)
            make_identity(nc, ident[:])
            ident82 = cp.tile([P, 2, P], F8)     # identity pair for DR d-red
            nc.scalar.copy(ident82[:, 0, :], ident[:])
            nc.scalar.copy(ident82[:, 1, :], ident[:])
            zf82 = cp.tile([P, 2, P], F8)
            nc.vector.memset(zf82[:], 0.0)
            if has_bq or has_bv or has_bk or has_bc:
                ones1 = cp.tile([1, P], BF)
                nc.vector.memset(ones1[:], 1.0)

            wt = {}
            _const_dmas = []
            for nm, t, dt_ in (("Wq", Wq8, F8), ("Wk", Wk8, F8),
                               ("Wv", Wv8, F8), ("Rv", Rv8, F8),
                               ("Wc", Wc16, BF)):
                s = cp.tile([P, 2, EMB], dt_, tag=f"w{nm}")
                _const_dmas.append((s, t))
                wt[nm] = s
            invc_sb = cp.tile([P, NB], FP)
            _const_dmas.append((invc_sb, invc_d))
            if has_bq or has_bv:
                bqv_sb = cp.tile([1, 2 * EMB], BF, tag="bqv")
                nc.sync.dma_start(bqv_sb[:], bqv_d[:])
            if has_bk:
                bk_sb = cp.tile([1, EMB], BF, tag="bk")
                nc.sync.dma_start(bk_sb[:], bk_d[:])
                cnt_sb = cp.tile([1, NPC_PAD], BF)
                nc.sync.dma_start(cnt_sb[:], cnt_d[:])
            if has_bc:
                bc_sb = cp.tile([1, EMB], BF, tag="bc")
                nc.sync.dma_start(bc_sb[:], bc_d[:])

            st = {}

            # ---------------- stages (software pipeline over block PAIRS)
            # Post-S1 stages process a pair of blocks per op: ACT/DVE/Pool
            # per-op init costs are paid once per pair, and PSUM pair-tiles
            # use first-writer-zeroes (start=True only on each bank's first
            # matmul group; later groups accumulate onto the pending-zero
            # bytes, the same HW-proven idiom as the S1 window resets).
            def S0(m):      # SP: fetch pair m (blocks 2m, 2m+1)
                b = 2 * m
                qvr = ql.tile([P, 2, 6, P], F8, tag="qvr")
                hi = min(b + 2, NB)
                nc.sync.dma_start(qvr[:, 0:hi - b, :, :], qvr_d[:, b:hi, :, :])
                ko = kp.tile([P, KOW2MAX], F8, tag="ko")
                import os as _o
                if _o.environ.get("KSPLIT", "0") == "1":
                    mid = ko_off[min(b + 1, hi)]
                    nc.sync.dma_start(ko[:, 0:mid - ko_off[b]],
                                      ko_d[:, ko_off[b]:mid])
                    if mid < ko_off[hi]:
                        nc.sync.dma_start(
                            ko[:, mid - ko_off[b]:ko_off[hi] - ko_off[b]],
                            ko_d[:, mid:ko_off[hi]])
                else:
                    w = ko_off[hi] - ko_off[b]
                    nc.sync.dma_start(ko[:, 0:w],
                                      ko_d[:, ko_off[b]:ko_off[hi]])
                st["qvr", m] = qvr
                st["ko", m] = ko

            def blocks_of(m):
                b0 = 2 * m
                return [b0] if b0 + 1 >= NB else [b0, b0 + 1]

            def S1(m):      # PE: U^T pair accumulation (DR, shared one-hot)
                ko = st.pop(("ko", m))
                blks = blocks_of(m)
                ps_uT = pu.tile([P, 2, 2, P], FP, space="PSUM", tag="uT")
                # full-width zero reset (write-mode), then accumulate
                mms = []
                for j in range(2):
                    for hf in range(2):
                        mms.append(dict(out=ps_uT[:, j, hf, :], lhsT=zf82[:],
                                        rhs=zf82[:], start=True, stop=False,
                                        perf_mode=DR, skip_group_check=True))
                for b in blks:
                    base = ko_off[b] - ko_off[2 * m]
                    T = int(pairs_per_block[b])
                    ke = ko[:, base:base + T * 2 * EMB].rearrange(
                        "p (t two c) -> p t two c", two=2, c=EMB)
                    oh = ko[:, base + T * 2 * EMB:base + T * 2 * EMB + SW[b]]
                    wo = 0
                    for t in range(T):
                        doff, W = windows[b][t]
                        if W > 0:
                            ohb = oh[:, wo:wo + W].unsqueeze(1).to_broadcast(
                                [P, 2, W])
                            for hf in range(2):
                                mms.append(dict(
                                    out=ps_uT[:, b % 2, hf, doff:doff + W],
                                    lhsT=ke[:, t, :, hf * P:(hf + 1) * P],
                                    rhs=ohb, start=False, stop=False,
                                    perf_mode=DR, skip_group_check=True))
                        wo += W
                    mms[-1]["stop"] = b == blks[-1]
                for kw in mms:
                    nc.tensor.matmul(**kw)
                st["ps_uT", m] = ps_uT

            def S1c(m):     # ACT op1: U^T pair -> SBUF fp8 (stays *SQ)
                ps_uT = st.pop(("ps_uT", m))
                uT_sb = wp.tile([P, 2, 2, P], F8, tag="uT_sb")
                nc.scalar.copy(uT_sb[:], ps_uT[:])
                st["uT_sb", m] = uT_sb

            def S2(m):      # PE: q/v/S projections, pair -> one x1024 region
                qvr = st.pop(("qvr", m))
                uT_sb = st.pop(("uT_sb", m))
                ps_qs = pqs.tile([P, 2, 3 * EMB], FP, space="PSUM", tag="qs")
                for j, b in enumerate(blocks_of(m)):
                    reg = ps_qs[:, j, :]
                    nc.tensor.matmul(out=reg[:, 0:EMB],
                                     lhsT=qvr[:, j, 0:2, :], rhs=wt["Wq"][:],
                                     start=True, stop=not has_bq,
                                     perf_mode=DR, skip_group_check=True)
                    if has_bq:
                        nc.tensor.matmul(out=reg[:, 0:EMB], lhsT=ones1[:],
                                         rhs=bqv_sb[:, 0:EMB],
                                         start=False, stop=True,
                                         skip_group_check=True)
                    nc.tensor.matmul(out=reg[:, EMB:2 * EMB],
                                     lhsT=qvr[:, j, 2:4, :], rhs=wt["Wv"][:],
                                     start=True, stop=False,
                                     perf_mode=DR, skip_group_check=True)
                    nc.tensor.matmul(out=reg[:, EMB:2 * EMB],
                                     lhsT=qvr[:, j, 2:4, :], rhs=wt["Rv"][:],
                                     start=False, stop=False,
                                     perf_mode=DR, skip_group_check=True)
                    nc.tensor.matmul(out=reg[:, EMB:2 * EMB],
                                     lhsT=qvr[:, j, 4:6, :], rhs=wt["Wv"][:],
                                     start=False, stop=not has_bv,
                                     perf_mode=DR, skip_group_check=True)
                    if has_bv:
                        nc.tensor.matmul(out=reg[:, EMB:2 * EMB],
                                         lhsT=ones1[:],
                                         rhs=bqv_sb[:, EMB:2 * EMB],
                                         start=False, stop=True,
                                         skip_group_check=True)
                    nc.tensor.matmul(out=reg[:, 2 * EMB:3 * EMB],
                                     lhsT=uT_sb[:, j, :, :], rhs=wt["Wk"][:],
                                     start=True, stop=not has_bk,
                                     perf_mode=DR, skip_group_check=True)
                    if has_bk:
                        nc.tensor.matmul(
                            out=reg[:, 2 * EMB:3 * EMB],
                            lhsT=cnt_sb[:, b * P:(b + 1) * P],
                            rhs=bk_sb[:], start=False, stop=True,
                            skip_group_check=True)
                st["ps_qs", m] = ps_qs

            def S2c(m):     # ACT op2: merged qh/vh/S pair copy
                ps_qs = st.pop(("ps_qs", m))
                qs_sb = ql.tile([P, 2, 3 * EMB], BF, tag="qs_sb")
                nc.scalar.mul(qs_sb[:], ps_qs[:], 1.0 / SPROJ)
                st["qs_sb", m] = qs_sb

            def S3b(m):     # DVE: prod bf16; Pool: fp8 g-tail (per half
                            # -- ISA free-dim patterns are 3D max)
                qs_sb = st[("qs_sb", m)]
                gs = H - PG_PROD
                prod = pr.tile([P, 2, H, gs, D], BF, tag="prod")
                prod8 = pr.tile([P, 2, H, PG_PROD, D], F8, tag="prod8")
                for j in range(2):
                    qh = qs_sb[:, j, 0:EMB].rearrange(
                        "p (h d) -> p h d", h=H)
                    sg = qs_sb[:, j, 2 * EMB:3 * EMB].rearrange(
                        "p (g d) -> p g d", g=H)
                    nc.vector.tensor_tensor(
                        out=prod[:, j],
                        in0=qh.unsqueeze(2).to_broadcast([P, H, gs, D]),
                        in1=sg[:, 0:gs, :].unsqueeze(1).to_broadcast(
                            [P, H, gs, D]),
                        op=mybir.AluOpType.mult)
                    nc.gpsimd.tensor_tensor(
                        out=prod8[:, j],
                        in0=qh.unsqueeze(2).to_broadcast(
                            [P, H, PG_PROD, D]),
                        in1=sg[:, gs:H, :].unsqueeze(1).to_broadcast(
                            [P, H, PG_PROD, D]),
                        op=mybir.AluOpType.mult)
                st["prod", m] = prod
                st["prod8", m] = prod8

            def S3p(m):     # PE: pair d-red (32 bf16 + 2x16 fp8-DR)
                prod = st.pop(("prod", m))
                prod8 = st.pop(("prod8", m))
                gs = H - PG_PROD
                ps_sc = psc.tile([P, 2, H, H], FP, space="PSUM", tag="sc")
                for dd in range(D):
                    nc.tensor.matmul(out=ps_sc[:, :, :, 0:gs], lhsT=ident[:],
                                     rhs=prod[:, :, :, :, dd],
                                     start=(dd == 0), stop=False,
                                     skip_group_check=True)
                for j in range(2):
                    for jj in range(D // 2):
                        nc.tensor.matmul(
                            out=ps_sc[:, j, :, gs:H],
                            lhsT=ident82[:],
                            rhs=prod8[:, j, :, :, 2 * jj:2 * jj + 2
                                      ].rearrange("p h g d -> p d (h g)"),
                            start=(jj == 0),
                            stop=(jj == D // 2 - 1) and j == 1,
                            perf_mode=DR, skip_group_check=True)
                st["ps_sc", m] = ps_sc

            def S3f(m):     # ACT (last): per-block exp from PSUM
                ps_sc = st.pop(("ps_sc", m))
                ex = wp.tile([P, 2, H, H], BF, tag="ex")
                for j, b in enumerate(blocks_of(m)):
                    nc.scalar.activation(
                        out=ex[:, j, :, :], in_=ps_sc[:, j, :, :],
                        func=mybir.ActivationFunctionType.Exp,
                        scale=invc_sb[:, b:b + 1])
                st["ex", m] = ex

            def S4r(m):     # DVE: pair den + rden; Pool: att = ex * rden
                ex = st.pop(("ex", m))
                den = wp.tile([P, 2, H], FP, tag="den")
                nc.vector.tensor_reduce(den[:], ex[:],
                                        axis=mybir.AxisListType.X,
                                        op=mybir.AluOpType.add)
                rden = wp.tile([P, 2, H], BF, tag="rden")
                with nc.allow_low_precision(
                        reason="bf16 softmax normalization, 2^-8 rel err"):
                    nc.vector.reciprocal(rden[:], den[:])
                att = wp.tile([P, 2, H, H], BF, tag="att")
                nc.gpsimd.tensor_tensor(
                    out=att[:], in0=ex[:],
                    in1=rden[:].unsqueeze(3).to_broadcast([P, 2, H, H]),
                    op=mybir.AluOpType.mult)
                st["att", m] = att

            def S4b(m):     # DVE+Pool: p2 = att (x) vh (per half)
                qs_sb = st.pop(("qs_sb", m))
                att = st.pop(("att", m))
                p2 = pr.tile([P, 2, H, D, H], BF, tag="p2")
                gs = H - PG_P2
                for j in range(2):
                    vh = qs_sb[:, j, EMB:2 * EMB].rearrange(
                        "p (d g) -> p d g", d=D)
                    nc.vector.tensor_tensor(
                        out=p2[:, j, :, :, 0:gs],
                        in0=att[:, j, :, 0:gs].unsqueeze(2).to_broadcast(
                            [P, H, D, gs]),
                        in1=vh[:, :, 0:gs].unsqueeze(1).to_broadcast(
                            [P, H, D, gs]),
                        op=mybir.AluOpType.mult)
                    nc.gpsimd.tensor_tensor(
                        out=p2[:, j, :, :, gs:H],
                        in0=att[:, j, :, gs:H].unsqueeze(2).to_broadcast(
                            [P, H, D, PG_P2]),
                        in1=vh[:, :, gs:H].unsqueeze(1).to_broadcast(
                            [P, H, D, PG_P2]),
                        op=mybir.AluOpType.mult)
                st["p2", m] = p2

            def S5a(m):     # PE: pair transposing g-sum -> ov^T in PSUM
                p2 = st.pop(("p2", m))
                ps_ov = pov.tile([P, 2, 2, P], FP, space="PSUM", tag="ov")
                for j, b in enumerate(blocks_of(m)):
                    for hf in range(2):
                        for g in range(H):
                            lhsT = p2[:, j, 4 * hf:4 * hf + 4, :, g
                                      ].rearrange("p h d -> p (h d)")
                            nc.tensor.matmul(
                                out=ps_ov[:, j, hf, :], lhsT=lhsT,
                                rhs=ident[:],
                                start=(g == 0), stop=(g == H - 1),
                                skip_group_check=True)
                st["ps_ov", m] = ps_ov

            def S5b(m):     # ACT op3: ov^T pair -> SBUF bf16
                ps_ov = st.pop(("ps_ov", m))
                ovT = wp.tile([P, 2, 2, P], BF, tag="ovT")
                nc.scalar.copy(ovT[:], ps_ov[:])
                st["ovT", m] = ovT

            def S5c(m):     # PE: pair out^T = Wc^T @ ov^T (bf16)
                ovT = st.pop(("ovT", m))
                ps_o = po.tile([P, 2, 2, P], FP, space="PSUM", tag="o")
                for j, b in enumerate(blocks_of(m)):
                    for cf in range(2):
                        for hh in range(2):
                            nc.tensor.matmul(
                                out=ps_o[:, j, cf, :],
                                lhsT=wt["Wc"][:, hh, cf * P:(cf + 1) * P],
                                rhs=ovT[:, j, hh, :],
                                start=(hh == 0),
                                stop=(hh == 1) and not has_bc,
                                skip_group_check=True)
                        if has_bc:
                            nc.tensor.matmul(
                                out=ps_o[:, j, cf, :],
                                lhsT=bc_sb[:, cf * P:(cf + 1) * P],
                                rhs=ones1[:],
                                start=False, stop=True,
                                skip_group_check=True)
                st["ps_o", m] = ps_o

            def S5d(m):     # ACT op4: fp16 pair out; SP: DMA pair
                ps_o = st.pop(("ps_o", m))
                fo = ql.tile([P, 2, 2, P], F16, tag="fo")
                nc.scalar.copy(fo[:], ps_o[:])
                b = 2 * m
                hi = min(b + 2, NB)
                nc.sync.dma_start(outT_d[:, b:hi, :, :], fo[:, 0:hi - b, :, :])

            # list order = per-engine priority order; exp (S3f) last so its
            # wait on the same-iteration d-reduce doesn't delay the copies
            import os
            _p = os.environ.get("KPERM", "7")
            if _p == "0":
                stages = [(0, S0), (3, S2), (1, S1), (2, S1c), (3, S2c),
                          (4, S3b), (5, S3p), (6, S4r), (7, S4b), (8, S5a),
                          (9, S5b), (9, S5c), (10, S5d), (5, S3f)]
            elif _p == "1":   # copies earlier in priority
                stages = [(0, S0), (2, S1c), (3, S2c), (9, S5b), (3, S2),
                          (1, S1), (4, S3b), (5, S3p), (6, S4r), (7, S4b),
                          (8, S5a), (9, S5c), (10, S5d), (5, S3f)]
            elif _p == "2":   # tail stages high priority
                stages = [(0, S0), (9, S5b), (9, S5c), (10, S5d), (8, S5a),
                          (7, S4b), (6, S4r), (5, S3p), (4, S3b), (3, S2),
                          (3, S2c), (2, S1c), (1, S1), (5, S3f)]
            elif _p == "4":   # tail-first but copies before compute
                stages = [(0, S0), (10, S5d), (9, S5b), (2, S1c), (3, S2c),
                          (9, S5c), (8, S5a), (7, S4b), (6, S4r), (5, S3p),
                          (4, S3b), (3, S2), (1, S1), (5, S3f)]
            elif _p == "5":   # S0 after tail stages
                stages = [(10, S5d), (9, S5b), (9, S5c), (0, S0), (8, S5a),
                          (7, S4b), (6, S4r), (5, S3p), (4, S3b), (3, S2),
                          (3, S2c), (2, S1c), (1, S1), (5, S3f)]
            elif _p == "7":
                stages = [(0, S0), (10, S5d), (9, S5b), (9, S5c), (8, S5a),
                          (7, S4b), (6, S4r), (5, S3p), (5, S3f), (4, S3b),
                          (3, S2), (3, S2c), (2, S1c), (1, S1)]
            elif _p == "8":
                stages = [(0, S0), (5, S3p), (5, S3f), (9, S5b), (9, S5c),
                          (10, S5d), (8, S5a), (7, S4b), (6, S4r), (4, S3b),
                          (3, S2), (3, S2c), (2, S1c), (1, S1)]
            elif _p == "3":   # DVE/Pool work first
                stages = [(0, S0), (4, S3b), (7, S4b), (6, S4r), (3, S2),
                          (1, S1), (2, S1c), (3, S2c), (5, S3p), (8, S5a),
                          (9, S5b), (9, S5c), (10, S5d), (5, S3f)]
            DEPTH = 11
            import os as _os
            _pf = _os.environ.get("KPF", "1") == "1"
            NP2 = (NB + 1) // 2
            for i in range(NP2 + DEPTH - 1):
                if i == 1:
                    # consts issue after the first input prefetches so the
                    # first ko stream heads the DMA queue (needed @ iter 3)
                    for s_, t_ in _const_dmas:
                        nc.sync.dma_start(s_[:], t_[:])
                for off, fn in stages:
                    if fn is S0 and _pf:
                        if i == 0:
                            fn(0)
                        if i + 1 < NP2:
                            fn(i + 1)
                        continue
                    mm = i - off
                    if 0 <= mm < NP2:
                        fn(mm)

    if split_waits:
        _split_sync_waits(nc)
    return nc


# --------------------------------------------------------------- host prep
def _prep(q, k, v, edge_index, Wq, bq, Wk, bk, Wv, bv, Wc, bc):
    A = np.asarray(edge_index[0], dtype=np.int64)
    B = np.asarray(edge_index[1], dtype=np.int64)
    order = np.argsort(A, kind="stable")
    A_s = A[order]
    B_s = B[order]

    core_lo = np.searchsorted(A_s, np.arange(NCORES) * NPC, side="left")
    core_hi = np.searchsorted(A_s, (np.arange(NCORES) + 1) * NPC, side="left")

    # --- per-core 2-deep slot assignment (vectorized)
    per_core = []
    npair = np.zeros((NCORES, NB), dtype=np.int64)
    for o in range(NCORES):
        a = A_s[core_lo[o]:core_hi[o]] - o * NPC      # local dest, ascending
        bi = B_s[core_lo[o]:core_hi[o]]
        n = len(a)
        first = np.searchsorted(a, a, side="left")
        rank = np.arange(n) - first
        depth = rank % 2
        cnt_d = np.bincount(a, minlength=NPC_PAD)
        s_d = (cnt_d + 1) // 2
        s_cum = np.cumsum(s_d) - s_d                   # global slot prefix
        blk_start = s_cum[np.arange(NB) * P]           # first slot of block
        slot_in_blk = (s_cum[a] - blk_start[a // P]) + rank // 2
        t = slot_in_blk // P
        p = slot_in_blk % P
        blk = a // P
        np.maximum.at(npair[o], blk, t + 1)
        per_core.append((a, bi, blk, t, p, depth))
    pairs_per_block = np.maximum(1, npair.max(axis=0)).astype(int)

    # --- per-(block,tile) destination windows, union across cores
    TMAX = int(pairs_per_block.max())
    lo = np.full((NB, TMAX), P, dtype=np.int64)
    hi = np.full((NB, TMAX), -1, dtype=np.int64)
    for o in range(NCORES):
        a, bi, blk, t, p, depth = per_core[o]
        dl = a - blk * P
        np.minimum.at(lo, (blk, t), dl)
        np.maximum.at(hi, (blk, t), dl)
    windows = []
    for bidx in range(NB):
        wb = []
        for t in range(int(pairs_per_block[bidx])):
            if hi[bidx, t] < 0:
                wb.append((0, 0))
                continue
            doff = int(lo[bidx, t])
            W = int(hi[bidx, t]) - doff + 1
            W = min((W + 3) // 4 * 4, P - doff)
            wb.append((doff, W))
        windows.append(wb)
    SW = [sum(w for _, w in wb) for wb in windows]

    KOW = [int(pairs_per_block[b]) * 2 * EMB + SW[b] for b in range(NB)]
    ko_off = np.zeros(NB + 1, dtype=np.int64)
    ko_off[1:] = np.cumsum(KOW)
    ke_base = ko_off[:NB]
    oh_base = ko_off[:NB] + pairs_per_block * 2 * EMB
    oh_col = np.zeros((NB, TMAX), dtype=np.int64)
    doffs = np.zeros((NB, TMAX), dtype=np.int64)
    for bidx in range(NB):
        acc = 0
        for t in range(int(pairs_per_block[bidx])):
            oh_col[bidx, t] = acc
            acc += windows[bidx][t][1]
            doffs[bidx, t] = windows[bidx][t][0]

    k8 = (np.asarray(k, np.float32) * SQ).astype(NP_F8)
    kos = []
    for o in range(NCORES):
        a, bi, blk, t, p, depth = per_core[o]
        ko = np.zeros((P, int(ko_off[-1])), dtype=NP_F8)
        cstart = ke_base[blk] + t * 2 * EMB + depth * EMB
        cidx = cstart[:, None] + np.arange(EMB)[None, :]
        ko[p[:, None], cidx] = k8[bi]
        m0 = depth == 0
        cols = (oh_base[blk[m0]] + oh_col[blk[m0], t[m0]]
                + (a[m0] - blk[m0] * P - doffs[blk[m0], t[m0]]))
        ko[p[m0], cols] = 1.0
        kos.append(ko)

    cnt_nodes = np.bincount(A, minlength=N_NODES).astype(np.float32)
    invc_full = 1.0 / np.maximum(cnt_nodes, 1.0)
    invcs, cnts = [], []
    for o in range(NCORES):
        s = np.ones(NPC_PAD, dtype=np.float32)
        s[:NPC] = invc_full[o * NPC:(o + 1) * NPC]
        invcs.append(np.ascontiguousarray(s.reshape(NB, P).T))
        c = np.zeros(NPC_PAD, dtype=np.float32)
        c[:NPC] = cnt_nodes[o * NPC:(o + 1) * NPC]
        cnts.append((c * SPROJ).reshape(1, NPC_PAD).astype(NP_BF))

    # q fp8; v as fp8 value + fp8 residual (both *SQ, exact power-of-2)
    q8 = (np.asarray(q, np.float32) * SQ).astype(NP_F8)
    vs = np.asarray(v, np.float32) * SQ
    v8 = vs.astype(NP_F8)
    r8 = (vs - v8.astype(np.float32)).astype(NP_F8)
    qvrs = []
    for o in range(NCORES):
        qvr = np.zeros((P, NB, 6, P), dtype=NP_F8)
        for j, src in ((0, q8), (2, v8), (4, r8)):
            sT = np.zeros((EMB, NPC_PAD), dtype=NP_F8)
            sT[:, :NPC] = src[o * NPC:(o + 1) * NPC].T
            qvr[:, :, j, :] = sT[0:P].reshape(P, NB, P)
            qvr[:, :, j + 1, :] = sT[P:EMB].reshape(P, NB, P)
        qvrs.append(qvr)

    # Wv column permutation: vh lands as [n, (d, g)]
    WvT = np.ascontiguousarray(np.asarray(Wv, np.float32).T)
    WvT_perm = WvT.reshape(EMB, H, D).transpose(0, 2, 1).reshape(EMB, EMB)
    bv_perm = np.asarray(bv, np.float32).reshape(H, D).T.reshape(-1)

    def pack2(WT):
        # [ch, out] -> [ch%128, ch//128, out]
        a = np.ascontiguousarray(np.asarray(WT, np.float32)).reshape(
            2, P, EMB)
        return np.ascontiguousarray(a.transpose(1, 0, 2))

    Wv_s = WvT_perm * SW_
    Wv8 = Wv_s.astype(NP_F8)
    Rv8 = (Wv_s - Wv8.astype(np.float32)).astype(NP_F8)

    bias_flags = (bool(np.any(np.asarray(bq))), bool(np.any(np.asarray(bk))),
                  bool(np.any(np.asarray(bv))), bool(np.any(np.asarray(bc))))
    has_bq, has_bk, has_bv, has_bc = bias_flags

    com = {
        "Wq8": pack2(np.asarray(Wq, np.float32).T * SW_).astype(NP_F8),
        "Wk8": pack2(np.asarray(Wk, np.float32).T * SW_).astype(NP_F8),
        "Wv8": pack2(Wv8.astype(np.float32)).astype(NP_F8),
        "Rv8": pack2(Rv8.astype(np.float32)).astype(NP_F8),
        "Wc16": pack2(np.asarray(Wc, np.float32).T).astype(NP_BF),
    }
    if has_bq or has_bv:
        bqv = np.concatenate([np.asarray(bq, np.float32),
                              bv_perm]) * SPROJ
        com["bqv"] = bqv.reshape(1, 2 * EMB).astype(NP_BF)
    if has_bk:
        com["bk"] = np.asarray(bk, np.float32).reshape(1, EMB).astype(NP_BF)
    if has_bc:
        com["bc"] = np.asarray(bc, np.float32).reshape(1, EMB).astype(NP_BF)

    in_maps = []
    for o in range(NCORES):
        m = dict(com)
        m["qvr"] = qvrs[o]
        m["ko"] = kos[o]
        m["invc"] = invcs[o]
        if has_bk:
            m["cnt"] = cnts[o]
        in_maps.append(m)
    return pairs_per_block.tolist(), windows, bias_flags, in_maps


_LAST = {}


def kernel(q, k, v, edge_index, Wq, bq, Wk, bk, Wv, bv, Wc, bc, latent=None,
           _want_results=False, _trace=False):
    pairs_per_block, windows, bias_flags, in_maps = _prep(
        q, k, v, edge_index, Wq, bq, Wk, bk, Wv, bv, Wc, bc)
    key = str((pairs_per_block, windows, bias_flags))
    if _LAST.get("key") != key:
        _LAST["nc"] = build_nc(pairs_per_block, windows, bias_flags)
        _LAST["key"] = key
    nc = _LAST["nc"]

    res = run_bass_kernel_spmd(nc, in_maps, core_ids=list(range(NCORES)),
                               trace=_trace)
    out = np.empty((N_NODES, EMB), dtype=np.float32)
    for o in range(NCORES):
        oT = res.results[o]["outT"].astype(np.float32)   # [P, NB, 2, P]
        full = np.empty((EMB, NPC_PAD), dtype=np.float32)
        full[0:P] = oT[:, :, 0, :].reshape(P, NPC_PAD)
        full[P:EMB] = oT[:, :, 1, :].reshape(P, NPC_PAD)
        out[o * NPC:(o + 1) * NPC] = full[:, :NPC].T
    if _want_results:
        return out, res
    return out


# revision 41
# speedup vs baseline: 1.0246x; 1.0115x over previous
"""Trainium2 Bass kernel for GNN multi-head cross-attention message passing.

Math (see reference): per edge e: score[e,h,g] = qh[A[e],h,:] . kh[B[e],g,:]
segment-MEAN over destination A -> softmax over g -> att @ vh -> Wc projection.

Algebraic structure: sums[n,h,g] = qh[n,h,:] . S[n,g,:] with
S = (segment_sum of raw k rows) @ Wk^T, so the [E,H,H] score tensor is never
materialized and k is projected once per node after aggregation.  Nodes are
sharded contiguously across the 8 cores (edge lists sharded by destination),
so no collective is needed; the host gathers per-edge k rows into a
dest-sorted fp8 stream.

v5 design, tuned against the TimelineSim cost model (HW-validated
179327 -> 145378 ns, rel err 1.19e-2 vs 2e-2 budget):

Numerics: fp8 per-tensor noise (~3.6% rms) does NOT average away in matmuls,
so fp8 is used only where damped or residual-corrected:
 - score path (q, Wq, k, Wk, U^T roundtrip, and the Pool-written g-tail of
   the score products) is fp8: its noise is damped by the softmax argument
   |mean| ~ 0.1 to a ~0.4% output contribution
 - V path: v and Wv ship as fp8 VALUE + fp8 RESIDUAL pairs at matched
   power-of-2 scales (vh = v8@W8 + v8@Rw8 + r8@W8, ~0.6% error at DoubleRow
   speed); ov^T and Wc stay bf16; output is fp16

Structure (per core: 49 blocks of 128 destination nodes, processed as 25
block PAIRS through a depth-11 software pipeline):
 - edges are packed TWO-DEEP per destination: slot (row, pair-tile) holds up
   to 2 same-destination edges, so one host-built one-hot column drives an
   fp8 DoubleRow matmul whose moving operand broadcasts over the k-tile pair
   (0-stride dim): half the S1 matmuls and PE time of per-edge tiles at the
   same DMA bytes; per-tile destination WINDOWS keep the moving width ~20
 - q + v + v-residual ship in one fp8 stream (768B/part/block); k-rows +
   one-hots are fused per block; all streams DMA in 2-block chunks (the
   single HWDGE queue costs ~625ns per dma_start)
 - qh/vh/S projections accumulate into ONE [P, 2, 768] PSUM region at a
   common x1024 scale (U^T is copied at x8 so S lands x1024): one merged
   ACT copy per pair instead of three per block
 - score d-reduction: 32 identity matmuls (bf16 g-head) + 2x16 fp8
   DoubleRow ident-pair matmuls (g-tail) accumulating in PSUM; exp reads
   PSUM directly and is LAST in ACT's priority order
 - V-phase g-reduction runs on the PE as TRANSPOSING identity matmuls
   (lhsT = p2 g-slice, rhs = identity) accumulating ov^T[(h,d), n]: the
   g-sum and the output transpose are the same instructions, and the
   bf16 out-projection consumes ov^T directly; fp16 out^T DMAs per pair
   (host un-transposes for free)
 - softmax: den via DVE X-reduce, reciprocal on DVE (bf16), att-normalize
   on Pool before the V products (so no post-normalization)
 - elementwise products split DVE (g-head, bf16, 2x mode) / Pool (g-tail,
   fp8, feeding the DoubleRow d-reduce)
 - PSUM banks exactly 8: uT-pair(1) + qs-pair(3) + sc-pair(1) + ov-pair(2)
   + o-pair(1); pair tiles use write-mode start=True only on each region's
   first matmul group
 - engine busy/core: DMA 107us (edge stream 85us dominates, the serial
   floor of this memory-bound problem), PE 107us, DVE 97us, ACT 95us,
   Pool 91us
"""
import numpy as np
import ml_dtypes

import concourse.bass as bass
import concourse.mybir as mybir
import concourse.tile as tile
from concourse.bass_utils import run_bass_kernel_spmd
from concourse.masks import make_identity

# ---------------------------------------------------------------- constants
NCORES = 8
N_NODES = 50000
EMB = 256
H = 8
D = 32
P = 128

NPC = N_NODES // NCORES          # 6250 nodes per core
NB = (NPC + P - 1) // P          # 49 blocks of 128 nodes per core
NPC_PAD = NB * P                 # 6272

FP = mybir.dt.float32
BF = mybir.dt.bfloat16
F16 = mybir.dt.float16
F8 = mybir.dt.float8e4
DR = mybir.MatmulPerfMode.DoubleRow

NP_BF = ml_dtypes.bfloat16
NP_F8 = ml_dtypes.float8_e4m3fn

# power-of-2 fp8 scaling (exact): stream = true * SCALE
SQ = 8.0        # q, v (+ residual), k edge rows
SW_ = 128.0     # fp8 weight matrices
SPROJ = SQ * SW_   # common scale of the merged qh/vh/S PSUM region

# Pool g-shares of the two 2048-elem product tensors
PG_PROD = 2
PG_P2 = 1


# ------------------------------------------------------- sync-wait splitting
# The staged walrus accepts only ONE sync-wait command per instruction.
# Tile attaches several waits to some instructions.  Post-pass: hoist all but
# one wait of each over-limit instruction onto same-engine Drain carriers
# placed immediately before it (engine streams execute in block order, so
# "all waits hold before the instruction runs" is preserved).
_WS_COUNTER = [0]


def _split_sync_waits(nc, maxw=1):
    for f in nc.m.functions:
        for blk in f.blocks:
            insts = blk.instructions
            out = []
            changed = False
            for ins in insts:
                si = ins.sync_info
                if si is not None and len(si.on_wait) > maxw:
                    waits = list(si.on_wait)
                    k = len(waits) - maxw
                    for i in range(0, k, maxw):
                        _WS_COUNTER[0] += 1
                        d = mybir.InstDrain(
                            name=f"I-wsplit-{_WS_COUNTER[0]}", ins=[], outs=[]
                        )
                        d.engine = ins.engine
                        d.sync_info = mybir.SyncInfo(
                            on_wait=waits[i : i + maxw], on_update=[]
                        )
                        out.append(d)
                    si.on_wait = waits[k:]
                    changed = True
                out.append(ins)
            if changed:
                blk.instructions = out


# ------------------------------------------------------------- device kernel
def build_nc(pairs_per_block, windows, bias_flags, split_waits=True):
    """Build the SPMD Bass module.

    pairs_per_block[b] = 2-deep edge pair-tiles in block b (same across
    cores).  windows[b] = list of (doff, W) per pair-tile (cross-core
    union).  bias_flags = (has_bq, has_bk, has_bv, has_bc).
    """
    SW = [int(sum(w for _, w in wb)) for wb in windows]   # one-hot cols/block
    has_bq, has_bk, has_bv, has_bc = bias_flags

    nc = bass.Bass("TRN2", target_bir_lowering=False, debug=False,
                   num_devices=NCORES)

    # per-core inputs; qvr/ko/out DMA in 2-block pairs (single shared HWDGE
    # queue at ~625ns per dma_start)
    # qvr j-dim: (q8-lo, q8-hi, v8-lo, v8-hi, r8-lo, r8-hi), all *SQ
    qvr_d = nc.dram_tensor("qvr", [P, NB, 6, P], F8, kind="ExternalInput")
    KOW = [int(pairs_per_block[b]) * 2 * EMB + SW[b] for b in range(NB)]
    ko_d = nc.dram_tensor("ko", [P, sum(KOW)], F8, kind="ExternalInput")
    # fp8 weights [ch%128, ch//128, out] * SW_
    Wq8 = nc.dram_tensor("Wq8", [P, 2, EMB], F8, kind="ExternalInput")
    Wk8 = nc.dram_tensor("Wk8", [P, 2, EMB], F8, kind="ExternalInput")
    Wv8 = nc.dram_tensor("Wv8", [P, 2, EMB], F8, kind="ExternalInput")  # perm
    Rv8 = nc.dram_tensor("Rv8", [P, 2, EMB], F8, kind="ExternalInput")  # perm
    # bf16 Wc^T [(h,d)%128, (h,d)//128, c'] true scale
    Wc16 = nc.dram_tensor("Wc16", [P, 2, EMB], BF, kind="ExternalInput")
    invc_d = nc.dram_tensor("invc", [P, NB], FP, kind="ExternalInput")
    if has_bq or has_bv:
        bqv_d = nc.dram_tensor("bqv", [1, 2 * EMB], BF, kind="ExternalInput")
    if has_bk:
        bk_d = nc.dram_tensor("bk", [1, EMB], BF, kind="ExternalInput")
        cnt_d = nc.dram_tensor("cnt", [1, NPC_PAD], BF, kind="ExternalInput")
    if has_bc:
        bc_d = nc.dram_tensor("bc", [1, EMB], BF, kind="ExternalInput")

    outT_d = nc.dram_tensor("outT", [P, NB, 2, P], F16, kind="ExternalOutput")

    ko_off = [0]
    for b in range(NB):
        ko_off.append(ko_off[-1] + KOW[b])
    KOW2MAX = max(KOW[b] + (KOW[b + 1] if b + 1 < NB else 0)
                  for b in range(0, NB, 2))

    with tile.TileContext(nc) as tc:
        with (
            tc.tile_pool(name="const", bufs=1) as cp,
            tc.tile_pool(name="work", bufs=6) as wp,
            tc.tile_pool(name="qvl", bufs=10) as ql,
            tc.tile_pool(name="kep", bufs=5) as kp,
            tc.tile_pool(name="prd", bufs=4) as pr,
            tc.tile_pool(name="ps_u", bufs=1, space="PSUM") as pu,
            tc.tile_pool(name="ps_qs", bufs=1, space="PSUM") as pqs,
            tc.tile_pool(name="ps_sc", bufs=1, space="PSUM") as psc,
            tc.tile_pool(name="ps_ov", bufs=2, space="PSUM") as pov,
            tc.tile_pool(name="ps_o", bufs=1, space="PSUM") as po,
        ):
            # ---------------- constants
            ident = cp.tile([P, P], BF)
            make_identity(nc, ident[:])
            ident82 = cp.tile([P, 2, P], F8)     # identity pair for DR d-red
            nc.scalar.copy(ident82[:, 0, :], ident[:])
            nc.scalar.copy(ident82[:, 1, :], ident[:])
            zf82 = cp.tile([P, 2, P], F8)
            nc.vector.memset(zf82[:], 0.0)
            if has_bq or has_bv or has_bk or has_bc:
                ones1 = cp.tile([1, P], BF)
                nc.vector.memset(ones1[:], 1.0)

            wt = {}
            _const_dmas = []
            for nm, t, dt_ in (("Wq", Wq8, F8), ("Wk", Wk8, F8),
                               ("Wv", Wv8, F8), ("Rv", Rv8, F8),
                               ("Wc", Wc16, BF)):
                s = cp.tile([P, 2, EMB], dt_, tag=f"w{nm}")
                _const_dmas.append((s, t))
                wt[nm] = s
            invc_sb = cp.tile([P, NB], FP)
            _const_dmas.append((invc_sb, invc_d))
            if has_bq or has_bv:
                bqv_sb = cp.tile([1, 2 * EMB], BF, tag="bqv")
                nc.sync.dma_start(bqv_sb[:], bqv_d[:])
            if has_bk:
                bk_sb = cp.tile([1, EMB], BF, tag="bk")
                nc.sync.dma_start(bk_sb[:], bk_d[:])
                cnt_sb = cp.tile([1, NPC_PAD], BF)
                nc.sync.dma_start(cnt_sb[:], cnt_d[:])
            if has_bc:
                bc_sb = cp.tile([1, EMB], BF, tag="bc")
                nc.sync.dma_start(bc_sb[:], bc_d[:])

            st = {}

            # ---------------- stages (software pipeline over block PAIRS)
            # Post-S1 stages process a pair of blocks per op: ACT/DVE/Pool
            # per-op init costs are paid once per pair, and PSUM pair-tiles
            # use first-writer-zeroes (start=True only on each bank's first
            # matmul group; later groups accumulate onto the pending-zero
            # bytes, the same HW-proven idiom as the S1 window resets).
            def S0(m):      # SP: fetch pair m (blocks 2m, 2m+1)
                b = 2 * m
                qvr = ql.tile([P, 2, 6, P], F8, tag="qvr")
                hi = min(b + 2, NB)
                nc.sync.dma_start(qvr[:, 0:hi - b, :, :], qvr_d[:, b:hi, :, :])
                ko = kp.tile([P, KOW2MAX], F8, tag="ko")
                import os as _o
                if _o.environ.get("KSPLIT", "0") == "1":
                    mid = ko_off[min(b + 1, hi)]
                    nc.sync.dma_start(ko[:, 0:mid - ko_off[b]],
                                      ko_d[:, ko_off[b]:mid])
                    if mid < ko_off[hi]:
                        nc.sync.dma_start(
                            ko[:, mid - ko_off[b]:ko_off[hi] - ko_off[b]],
                            ko_d[:, mid:ko_off[hi]])
                else:
                    w = ko_off[hi] - ko_off[b]
                    nc.sync.dma_start(ko[:, 0:w],
                                      ko_d[:, ko_off[b]:ko_off[hi]])
                st["qvr", m] = qvr
                st["ko", m] = ko

            def blocks_of(m):
                b0 = 2 * m
                return [b0] if b0 + 1 >= NB else [b0, b0 + 1]

            def S1(m):      # PE: U^T pair accumulation (DR, shared one-hot)
                ko = st.pop(("ko", m))
                blks = blocks_of(m)
                ps_uT = pu.tile([P, 2, 2, P], FP, space="PSUM", tag="uT")
                # full-width zero reset (write-mode), then accumulate
                mms = []
                for j in range(2):
                    for hf in range(2):
                        mms.append(dict(out=ps_uT[:, j, hf, :], lhsT=zf82[:],
                                        rhs=zf82[:], start=True, stop=False,
                                        perf_mode=DR, skip_group_check=True))
                for b in blks:
                    base = ko_off[b] - ko_off[2 * m]
                    T = int(pairs_per_block[b])
                    ke = ko[:, base:base + T * 2 * EMB].rearrange(
                        "p (t two c) -> p t two c", two=2, c=EMB)
                    oh = ko[:, base + T * 2 * EMB:base + T * 2 * EMB + SW[b]]
                    wo = 0
                    for t in range(T):
                        doff, W = windows[b][t]
                        if W > 0:
                            ohb = oh[:, wo:wo + W].unsqueeze(1).to_broadcast(
                                [P, 2, W])
                            for hf in range(2):
                                mms.append(dict(
                                    out=ps_uT[:, b % 2, hf, doff:doff + W],
                                    lhsT=ke[:, t, :, hf * P:(hf + 1) * P],
                                    rhs=ohb, start=False, stop=False,
                                    perf_mode=DR, skip_group_check=True))
                        wo += W
                    mms[-1]["stop"] = b == blks[-1]
                for kw in mms:
                    nc.tensor.matmul(**kw)
                st["ps_uT", m] = ps_uT

            def S1c(m):     # ACT op1: U^T pair -> SBUF fp8 (stays *SQ)
                ps_uT = st.pop(("ps_uT", m))
                uT_sb = wp.tile([P, 2, 2, P], F8, tag="uT_sb")
                nc.scalar.copy(uT_sb[:], ps_uT[:])
                st["uT_sb", m] = uT_sb

            def S2(m):      # PE: q/v/S projections, pair -> one x1024 region
                qvr = st.pop(("qvr", m))
                uT_sb = st.pop(("uT_sb", m))
                ps_qs = pqs.tile([P, 2, 3 * EMB], FP, space="PSUM", tag="qs")
                for j, b in enumerate(blocks_of(m)):
                    reg = ps_qs[:, j, :]
                    nc.tensor.matmul(out=reg[:, 0:EMB],
                                     lhsT=qvr[:, j, 0:2, :], rhs=wt["Wq"][:],
                                     start=True, stop=not has_bq,
                                     perf_mode=DR, skip_group_check=True)
                    if has_bq:
                        nc.tensor.matmul(out=reg[:, 0:EMB], lhsT=ones1[:],
                                         rhs=bqv_sb[:, 0:EMB],
                                         start=False, stop=True,
                                         skip_group_check=True)
                    nc.tensor.matmul(out=reg[:, EMB:2 * EMB],
                                     lhsT=qvr[:, j, 2:4, :], rhs=wt["Wv"][:],
                                     start=True, stop=False,
                                     perf_mode=DR, skip_group_check=True)
                    nc.tensor.matmul(out=reg[:, EMB:2 * EMB],
                                     lhsT=qvr[:, j, 2:4, :], rhs=wt["Rv"][:],
                                     start=False, stop=False,
                                     perf_mode=DR, skip_group_check=True)
                    nc.tensor.matmul(out=reg[:, EMB:2 * EMB],
                                     lhsT=qvr[:, j, 4:6, :], rhs=wt["Wv"][:],
                                     start=False, stop=not has_bv,
                                     perf_mode=DR, skip_group_check=True)
                    if has_bv:
                        nc.tensor.matmul(out=reg[:, EMB:2 * EMB],
                                         lhsT=ones1[:],
                                         rhs=bqv_sb[:, EMB:2 * EMB],
                                         start=False, stop=True,
                                         skip_group_check=True)
                    nc.tensor.matmul(out=reg[:, 2 * EMB:3 * EMB],
                                     lhsT=uT_sb[:, j, :, :], rhs=wt["Wk"][:],
                                     start=True, stop=not has_bk,
                                     perf_mode=DR, skip_group_check=True)
                    if has_bk:
                        nc.tensor.matmul(
                            out=reg[:, 2 * EMB:3 * EMB],
                            lhsT=cnt_sb[:, b * P:(b + 1) * P],
                            rhs=bk_sb[:], start=False, stop=True,
                            skip_group_check=True)
                st["ps_qs", m] = ps_qs

            def S2c(m):     # ACT op2: merged qh/vh/S pair copy
                ps_qs = st.pop(("ps_qs", m))
                qs_sb = ql.tile([P, 2, 3 * EMB], BF, tag="qs_sb")
                nc.scalar.mul(qs_sb[:], ps_qs[:], 1.0 / SPROJ)
                st["qs_sb", m] = qs_sb

            def S3b(m):     # DVE: prod bf16; Pool: fp8 g-tail (per half
                            # -- ISA free-dim patterns are 3D max)
                qs_sb = st[("qs_sb", m)]
                gs = H - PG_PROD
                prod = pr.tile([P, 2, H, gs, D], BF, tag="prod")
                prod8 = pr.tile([P, 2, H, PG_PROD, D], F8, tag="prod8")
                for j in range(2):
                    qh = qs_sb[:, j, 0:EMB].rearrange(
                        "p (h d) -> p h d", h=H)
                    sg = qs_sb[:, j, 2 * EMB:3 * EMB].rearrange(
                        "p (g d) -> p g d", g=H)
                    nc.vector.tensor_tensor(
                        out=prod[:, j],
                        in0=qh.unsqueeze(2).to_broadcast([P, H, gs, D]),
                        in1=sg[:, 0:gs, :].unsqueeze(1).to_broadcast(
                            [P, H, gs, D]),
                        op=mybir.AluOpType.mult)
                    nc.gpsimd.tensor_tensor(
                        out=prod8[:, j],
                        in0=qh.unsqueeze(2).to_broadcast(
                            [P, H, PG_PROD, D]),
                        in1=sg[:, gs:H, :].unsqueeze(1).to_broadcast(
                            [P, H, PG_PROD, D]),
                        op=mybir.AluOpType.mult)
                st["prod", m] = prod
                st["prod8", m] = prod8

            def S3p(m):     # PE: pair d-red (32 bf16 + 2x16 fp8-DR)
                prod = st.pop(("prod", m))
                prod8 = st.pop(("prod8", m))
                gs = H - PG_PROD
                ps_sc = psc.tile([P, 2, H, H], FP, space="PSUM", tag="sc")
                for dd in range(D):
                    nc.tensor.matmul(out=ps_sc[:, :, :, 0:gs], lhsT=ident[:],
                                     rhs=prod[:, :, :, :, dd],
                                     start=(dd == 0), stop=False,
                                     skip_group_check=True)
                for j in range(2):
                    for jj in range(D // 2):
                        nc.tensor.matmul(
                            out=ps_sc[:, j, :, gs:H],
                            lhsT=ident82[:],
                            rhs=prod8[:, j, :, :, 2 * jj:2 * jj + 2
                                      ].rearrange("p h g d -> p d (h g)"),
                            start=(jj == 0),
                            stop=(jj == D // 2 - 1) and j == 1,
                            perf_mode=DR, skip_group_check=True)
                st["ps_sc", m] = ps_sc

            def S3f(m):     # ACT (last): per-block exp from PSUM
                ps_sc = st.pop(("ps_sc", m))
                ex = wp.tile([P, 2, H, H], BF, tag="ex")
                for j, b in enumerate(blocks_of(m)):
                    nc.scalar.activation(
                        out=ex[:, j, :, :], in_=ps_sc[:, j, :, :],
                        func=mybir.ActivationFunctionType.Exp,
                        scale=invc_sb[:, b:b + 1])
                st["ex", m] = ex

            def S4r(m):     # DVE: pair den + rden; Pool: att = ex * rden
                ex = st.pop(("ex", m))
                den = wp.tile([P, 2, H], FP, tag="den")
                nc.vector.tensor_reduce(den[:], ex[:],
                                        axis=mybir.AxisListType.X,
                                        op=mybir.AluOpType.add)
                rden = wp.tile([P, 2, H], BF, tag="rden")
                with nc.allow_low_precision(
                        reason="bf16 softmax normalization, 2^-8 rel err"):
                    nc.vector.reciprocal(rden[:], den[:])
                att = wp.tile([P, 2, H, H], BF, tag="att")
                nc.gpsimd.tensor_tensor(
                    out=att[:], in0=ex[:],
                    in1=rden[:].unsqueeze(3).to_broadcast([P, 2, H, H]),
                    op=mybir.AluOpType.mult)
                st["att", m] = att

            def S4b(m):     # DVE+Pool: p2 = att (x) vh (per half)
                qs_sb = st.pop(("qs_sb", m))
                att = st.pop(("att", m))
                p2 = pr.tile([P, 2, H, D, H], BF, tag="p2")
                gs = H - PG_P2
                for j in range(2):
                    vh = qs_sb[:, j, EMB:2 * EMB].rearrange(
                        "p (d g) -> p d g", d=D)
                    nc.vector.tensor_tensor(
                        out=p2[:, j, :, :, 0:gs],
                        in0=att[:, j, :, 0:gs].unsqueeze(2).to_broadcast(
                            [P, H, D, gs]),
                        in1=vh[:, :, 0:gs].unsqueeze(1).to_broadcast(
                            [P, H, D, gs]),
                        op=mybir.AluOpType.mult)
                    nc.gpsimd.tensor_tensor(
                        out=p2[:, j, :, :, gs:H],
                        in0=att[:, j, :, gs:H].unsqueeze(2).to_broadcast(
                            [P, H, D, PG_P2]),
                        in1=vh[:, :, gs:H].unsqueeze(1).to_broadcast(
                            [P, H, D, PG_P2]),
                        op=mybir.AluOpType.mult)
                st["p2", m] = p2

            def S5a(m):     # PE: pair transposing g-sum -> ov^T in PSUM
                p2 = st.pop(("p2", m))
                ps_ov = pov.tile([P, 2, 2, P], FP, space="PSUM", tag="ov")
                for j, b in enumerate(blocks_of(m)):
                    for hf in range(2):
                        for g in range(H):
                            lhsT = p2[:, j, 4 * hf:4 * hf + 4, :, g
                                      ].rearrange("p h d -> p (h d)")
                            nc.tensor.matmul(
                                out=ps_ov[:, j, hf, :], lhsT=lhsT,
                                rhs=ident[:],
                                start=(g == 0), stop=(g == H - 1),
                                skip_group_check=True)
                st["ps_ov", m] = ps_ov

            def S5b(m):     # ACT op3: ov^T pair -> SBUF bf16
                ps_ov = st.pop(("ps_ov", m))
                ovT = wp.tile([P, 2, 2, P], BF, tag="ovT")
                nc.scalar.copy(ovT[:], ps_ov[:])
                st["ovT", m] = ovT

            def S5c(m):     # PE: pair out^T = Wc^T @ ov^T (bf16)
                ovT = st.pop(("ovT", m))
                ps_o = po.tile([P, 2, 2, P], FP, space="PSUM", tag="o")
                for j, b in enumerate(blocks_of(m)):
                    for cf in range(2):
                        for hh in range(2):
                            nc.tensor.matmul(
                                out=ps_o[:, j, cf, :],
                                lhsT=wt["Wc"][:, hh, cf * P:(cf + 1) * P],
                                rhs=ovT[:, j, hh, :],
                                start=(hh == 0),
                                stop=(hh == 1) and not has_bc,
                                skip_group_check=True)
                        if has_bc:
                            nc.tensor.matmul(
                                out=ps_o[:, j, cf, :],
                                lhsT=bc_sb[:, cf * P:(cf + 1) * P],
                                rhs=ones1[:],
                                start=False, stop=True,
                                skip_group_check=True)
                st["ps_o", m] = ps_o

            def S5d(m):     # ACT op4: fp16 pair out; SP: DMA pair
                ps_o = st.pop(("ps_o", m))
                fo = ql.tile([P, 2, 2, P], F16, tag="fo")
                nc.scalar.copy(fo[:], ps_o[:])
                b = 2 * m
                hi = min(b + 2, NB)
                nc.sync.dma_start(outT_d[:, b:hi, :, :], fo[:, 0:hi - b, :, :])

            # list order = per-engine priority order; exp (S3f) last so its
            # wait on the same-iteration d-reduce doesn't delay the copies
            import os
            _p = os.environ.get("KPERM", "7")
            if _p == "0":
                stages = [(0, S0), (3, S2), (1, S1), (2, S1c), (3, S2c),
                          (4, S3b), (5, S3p), (6, S4r), (7, S4b), (8, S5a),
                          (9, S5b), (9, S5c), (10, S5d), (5, S3f)]
            elif _p == "1":   # copies earlier in priority
                stages = [(0, S0), (2, S1c), (3, S2c), (9, S5b), (3, S2),
                          (1, S1), (4, S3b), (5, S3p), (6, S4r), (7, S4b),
                          (8, S5a), (9, S5c), (10, S5d), (5, S3f)]
            elif _p == "2":   # tail stages high priority
                stages = [(0, S0), (9, S5b), (9, S5c), (10, S5d), (8, S5a),
                          (7, S4b), (6, S4r), (5, S3p), (4, S3b), (3, S2),
                          (3, S2c), (2, S1c), (1, S1), (5, S3f)]
            elif _p == "4":   # tail-first but copies before compute
                stages = [(0, S0), (10, S5d), (9, S5b), (2, S1c), (3, S2c),
                          (9, S5c), (8, S5a), (7, S4b), (6, S4r), (5, S3p),
                          (4, S3b), (3, S2), (1, S1), (5, S3f)]
            elif _p == "5":   # S0 after tail stages
                stages = [(10, S5d), (9, S5b), (9, S5c), (0, S0), (8, S5a),
                          (7, S4b), (6, S4r), (5, S3p), (4, S3b), (3, S2),
                          (3, S2c), (2, S1c), (1, S1), (5, S3f)]
            elif _p == "7":
                stages = [(0, S0), (10, S5d), (9, S5b), (9, S5c), (8, S5a),
                          (7, S4b), (6, S4r), (5, S3p), (5, S3f), (4, S3b),
                          (3, S2), (3, S2c), (2, S1c), (1, S1)]
            elif _p == "8":
                stages = [(0, S0), (5, S3p), (5, S3f), (9, S5b), (9, S5c),
                          (10, S5d), (8, S5a), (7, S4b), (6, S4r), (4, S3b),
                          (3, S2), (3, S2c), (2, S1c), (1, S1)]
            elif _p == "3":   # DVE/Pool work first
                stages = [(0, S0), (4, S3b), (7, S4b), (6, S4r), (3, S2),
                          (1, S1), (2, S1c), (3, S2c), (5, S3p), (8, S5a),
                          (9, S5b), (9, S5c), (10, S5d), (5, S3f)]
            DEPTH = 11
            import os as _os
            _pf = _os.environ.get("KPF", "1") == "1"
            NP2 = (NB + 1) // 2
            for i in range(NP2 + DEPTH - 1):
                if i == 1:
                    # consts issue after the first input prefetches so the
                    # first ko stream heads the DMA queue (needed @ iter 3)
                    for s_, t_ in _const_dmas:
                        nc.sync.dma_start(s_[:], t_[:])
                for off, fn in stages:
                    if fn is S0 and _pf:
                        if i == 0:
                            fn(0)
                            fn(1)
                        if i + 2 < NP2:
                            fn(i + 2)
                        continue
                    mm = i - off
                    if 0 <= mm < NP2:
                        fn(mm)

    if split_waits:
        _split_sync_waits(nc)
    return nc


# --------------------------------------------------------------- host prep
def _prep(q, k, v, edge_index, Wq, bq, Wk, bk, Wv, bv, Wc, bc):
    A = np.asarray(edge_index[0], dtype=np.int64)
    B = np.asarray(edge_index[1], dtype=np.int64)
    order = np.argsort(A, kind="stable")
    A_s = A[order]
    B_s = B[order]

    core_lo = np.searchsorted(A_s, np.arange(NCORES) * NPC, side="left")
    core_hi = np.searchsorted(A_s, (np.arange(NCORES) + 1) * NPC, side="left")

    # --- per-core 2-deep slot assignment (vectorized)
    per_core = []
    npair = np.zeros((NCORES, NB), dtype=np.int64)
    for o in range(NCORES):
        a = A_s[core_lo[o]:core_hi[o]] - o * NPC      # local dest, ascending
        bi = B_s[core_lo[o]:core_hi[o]]
        n = len(a)
        first = np.searchsorted(a, a, side="left")
        rank = np.arange(n) - first
        depth = rank % 2
        cnt_d = np.bincount(a, minlength=NPC_PAD)
        s_d = (cnt_d + 1) // 2
        s_cum = np.cumsum(s_d) - s_d                   # global slot prefix
        blk_start = s_cum[np.arange(NB) * P]           # first slot of block
        slot_in_blk = (s_cum[a] - blk_start[a // P]) + rank // 2
        t = slot_in_blk // P
        p = slot_in_blk % P
        blk = a // P
        np.maximum.at(npair[o], blk, t + 1)
        per_core.append((a, bi, blk, t, p, depth))
    pairs_per_block = np.maximum(1, npair.max(axis=0)).astype(int)

    # --- per-(block,tile) destination windows, union across cores
    TMAX = int(pairs_per_block.max())
    lo = np.full((NB, TMAX), P, dtype=np.int64)
    hi = np.full((NB, TMAX), -1, dtype=np.int64)
    for o in range(NCORES):
        a, bi, blk, t, p, depth = per_core[o]
        dl = a - blk * P
        np.minimum.at(lo, (blk, t), dl)
        np.maximum.at(hi, (blk, t), dl)
    windows = []
    for bidx in range(NB):
        wb = []
        for t in range(int(pairs_per_block[bidx])):
            if hi[bidx, t] < 0:
                wb.append((0, 0))
                continue
            doff = int(lo[bidx, t])
            W = int(hi[bidx, t]) - doff + 1
            W = min((W + 3) // 4 * 4, P - doff)
            wb.append((doff, W))
        windows.append(wb)
    SW = [sum(w for _, w in wb) for wb in windows]

    KOW = [int(pairs_per_block[b]) * 2 * EMB + SW[b] for b in range(NB)]
    ko_off = np.zeros(NB + 1, dtype=np.int64)
    ko_off[1:] = np.cumsum(KOW)
    ke_base = ko_off[:NB]
    oh_base = ko_off[:NB] + pairs_per_block * 2 * EMB
    oh_col = np.zeros((NB, TMAX), dtype=np.int64)
    doffs = np.zeros((NB, TMAX), dtype=np.int64)
    for bidx in range(NB):
        acc = 0
        for t in range(int(pairs_per_block[bidx])):
            oh_col[bidx, t] = acc
            acc += windows[bidx][t][1]
            doffs[bidx, t] = windows[bidx][t][0]

    k8 = (np.asarray(k, np.float32) * SQ).astype(NP_F8)
    kos = []
    for o in range(NCORES):
        a, bi, blk, t, p, depth = per_core[o]
        ko = np.zeros((P, int(ko_off[-1])), dtype=NP_F8)
        cstart = ke_base[blk] + t * 2 * EMB + depth * EMB
        cidx = cstart[:, None] + np.arange(EMB)[None, :]
        ko[p[:, None], cidx] = k8[bi]
        m0 = depth == 0
        cols = (oh_base[blk[m0]] + oh_col[blk[m0], t[m0]]
                + (a[m0] - blk[m0] * P - doffs[blk[m0], t[m0]]))
        ko[p[m0], cols] = 1.0
        kos.append(ko)

    cnt_nodes = np.bincount(A, minlength=N_NODES).astype(np.float32)
    invc_full = 1.0 / np.maximum(cnt_nodes, 1.0)
    invcs, cnts = [], []
    for o in range(NCORES):
        s = np.ones(NPC_PAD, dtype=np.float32)
        s[:NPC] = invc_full[o * NPC:(o + 1) * NPC]
        invcs.append(np.ascontiguousarray(s.reshape(NB, P).T))
        c = np.zeros(NPC_PAD, dtype=np.float32)
        c[:NPC] = cnt_nodes[o * NPC:(o + 1) * NPC]
        cnts.append((c * SPROJ).reshape(1, NPC_PAD).astype(NP_BF))

    # q fp8; v as fp8 value + fp8 residual (both *SQ, exact power-of-2)
    q8 = (np.asarray(q, np.float32) * SQ).astype(NP_F8)
    vs = np.asarray(v, np.float32) * SQ
    v8 = vs.astype(NP_F8)
    r8 = (vs - v8.astype(np.float32)).astype(NP_F8)
    qvrs = []
    for o in range(NCORES):
        qvr = np.zeros((P, NB, 6, P), dtype=NP_F8)
        for j, src in ((0, q8), (2, v8), (4, r8)):
            sT = np.zeros((EMB, NPC_PAD), dtype=NP_F8)
            sT[:, :NPC] = src[o * NPC:(o + 1) * NPC].T
            qvr[:, :, j, :] = sT[0:P].reshape(P, NB, P)
            qvr[:, :, j + 1, :] = sT[P:EMB].reshape(P, NB, P)
        qvrs.append(qvr)

    # Wv column permutation: vh lands as [n, (d, g)]
    WvT = np.ascontiguousarray(np.asarray(Wv, np.float32).T)
    WvT_perm = WvT.reshape(EMB, H, D).transpose(0, 2, 1).reshape(EMB, EMB)
    bv_perm = np.asarray(bv, np.float32).reshape(H, D).T.reshape(-1)

    def pack2(WT):
        # [ch, out] -> [ch%128, ch//128, out]
        a = np.ascontiguousarray(np.asarray(WT, np.float32)).reshape(
            2, P, EMB)
        return np.ascontiguousarray(a.transpose(1, 0, 2))

    Wv_s = WvT_perm * SW_
    Wv8 = Wv_s.astype(NP_F8)
    Rv8 = (Wv_s - Wv8.astype(np.float32)).astype(NP_F8)

    bias_flags = (bool(np.any(np.asarray(bq))), bool(np.any(np.asarray(bk))),
                  bool(np.any(np.asarray(bv))), bool(np.any(np.asarray(bc))))
    has_bq, has_bk, has_bv, has_bc = bias_flags

    com = {
        "Wq8": pack2(np.asarray(Wq, np.float32).T * SW_).astype(NP_F8),
        "Wk8": pack2(np.asarray(Wk, np.float32).T * SW_).astype(NP_F8),
        "Wv8": pack2(Wv8.astype(np.float32)).astype(NP_F8),
        "Rv8": pack2(Rv8.astype(np.float32)).astype(NP_F8),
        "Wc16": pack2(np.asarray(Wc, np.float32).T).astype(NP_BF),
    }
    if has_bq or has_bv:
        bqv = np.concatenate([np.asarray(bq, np.float32),
                              bv_perm]) * SPROJ
        com["bqv"] = bqv.reshape(1, 2 * EMB).astype(NP_BF)
    if has_bk:
        com["bk"] = np.asarray(bk, np.float32).reshape(1, EMB).astype(NP_BF)
    if has_bc:
        com["bc"] = np.asarray(bc, np.float32).reshape(1, EMB).astype(NP_BF)

    in_maps = []
    for o in range(NCORES):
        m = dict(com)
        m["qvr"] = qvrs[o]
        m["ko"] = kos[o]
        m["invc"] = invcs[o]
        if has_bk:
            m["cnt"] = cnts[o]
        in_maps.append(m)
    return pairs_per_block.tolist(), windows, bias_flags, in_maps


_LAST = {}


def kernel(q, k, v, edge_index, Wq, bq, Wk, bk, Wv, bv, Wc, bc, latent=None,
           _want_results=False, _trace=False):
    pairs_per_block, windows, bias_flags, in_maps = _prep(
        q, k, v, edge_index, Wq, bq, Wk, bk, Wv, bv, Wc, bc)
    key = str((pairs_per_block, windows, bias_flags))
    if _LAST.get("key") != key:
        _LAST["nc"] = build_nc(pairs_per_block, windows, bias_flags)
        _LAST["key"] = key
    nc = _LAST["nc"]

    res = run_bass_kernel_spmd(nc, in_maps, core_ids=list(range(NCORES)),
                               trace=_trace)
    out = np.empty((N_NODES, EMB), dtype=np.float32)
    for o in range(NCORES):
        oT = res.results[o]["outT"].astype(np.float32)   # [P, NB, 2, P]
        full = np.empty((EMB, NPC_PAD), dtype=np.float32)
        full[0:P] = oT[:, :, 0, :].reshape(P, NPC_PAD)
        full[P:EMB] = oT[:, :, 1, :].reshape(P, NPC_PAD)
        out[o * NPC:(o + 1) * NPC] = full[:, :NPC].T
    if _want_results:
        return out, res
    return out


# revision 43
# speedup vs baseline: 1.0348x; 1.0100x over previous
"""Trainium2 Bass kernel for GNN multi-head cross-attention message passing.

Math (see reference): per edge e: score[e,h,g] = qh[A[e],h,:] . kh[B[e],g,:]
segment-MEAN over destination A -> softmax over g -> att @ vh -> Wc projection.

Algebraic structure: sums[n,h,g] = qh[n,h,:] . S[n,g,:] with
S = (segment_sum of raw k rows) @ Wk^T, so the [E,H,H] score tensor is never
materialized and k is projected once per node after aggregation.  Nodes are
sharded contiguously across the 8 cores (edge lists sharded by destination),
so no collective is needed; the host gathers per-edge k rows into a
dest-sorted fp8 stream.

v5 design, tuned against the TimelineSim cost model (HW-validated
179327 -> 142504 ns, rel err 1.19e-2 vs 2e-2 budget):

Numerics: fp8 per-tensor noise (~3.6% rms) does NOT average away in matmuls,
so fp8 is used only where damped or residual-corrected:
 - score path (q, Wq, k, Wk, U^T roundtrip, and the Pool-written g-tail of
   the score products) is fp8: its noise is damped by the softmax argument
   |mean| ~ 0.1 to a ~0.4% output contribution
 - V path: v and Wv ship as fp8 VALUE + fp8 RESIDUAL pairs at matched
   power-of-2 scales (vh = v8@W8 + v8@Rw8 + r8@W8, ~0.6% error at DoubleRow
   speed); ov^T and Wc stay bf16; output is fp16

Structure (per core: 49 blocks of 128 destination nodes, processed as 25
block PAIRS through a depth-11 software pipeline):
 - edges are packed TWO-DEEP per destination: slot (row, pair-tile) holds up
   to 2 same-destination edges, so one host-built one-hot column drives an
   fp8 DoubleRow matmul whose moving operand broadcasts over the k-tile pair
   (0-stride dim): half the S1 matmuls and PE time of per-edge tiles at the
   same DMA bytes; per-tile destination WINDOWS keep the moving width ~20
 - q + v + v-residual ship in one fp8 stream (768B/part/block); k-rows +
   one-hots are fused per block; all streams DMA in 2-block chunks (the
   single HWDGE queue costs ~625ns per dma_start)
 - qh/vh/S projections accumulate into ONE [P, 2, 768] PSUM region at a
   common x1024 scale (U^T is copied at x8 so S lands x1024): one merged
   ACT copy per pair instead of three per block
 - score d-reduction: 32 identity matmuls (bf16 g-head) + 2x16 fp8
   DoubleRow ident-pair matmuls (g-tail) accumulating in PSUM; exp reads
   PSUM directly and is LAST in ACT's priority order
 - V-phase g-reduction runs on the PE as TRANSPOSING identity matmuls
   (lhsT = p2 g-slice, rhs = identity) accumulating ov^T[(h,d), n]: the
   g-sum and the output transpose are the same instructions, and the
   bf16 out-projection consumes ov^T directly; fp16 out^T DMAs per pair
   (host un-transposes for free)
 - softmax: den via DVE X-reduce, reciprocal on DVE (bf16), att-normalize
   on Pool before the V products (so no post-normalization)
 - elementwise products split DVE (g-head, bf16, 2x mode) / Pool (g-tail,
   fp8, feeding the DoubleRow d-reduce)
 - PSUM banks exactly 8: uT-pair(1) + qs-pair(3) + sc-pair(1) + ov-pair(2)
   + o-pair(1); pair tiles use write-mode start=True only on each region's
   first matmul group
 - engine busy/core: DMA 107us (edge stream 85us dominates, the serial
   floor of this memory-bound problem), PE 107us, DVE 97us, ACT 95us,
   Pool 91us
"""
import numpy as np
import ml_dtypes

import concourse.bass as bass
import concourse.mybir as mybir
import concourse.tile as tile
from concourse.bass_utils import run_bass_kernel_spmd
from concourse.masks import make_identity

# ---------------------------------------------------------------- constants
NCORES = 8
N_NODES = 50000
EMB = 256
H = 8
D = 32
P = 128

NPC = N_NODES // NCORES          # 6250 nodes per core
NB = (NPC + P - 1) // P          # 49 blocks of 128 nodes per core
NPC_PAD = NB * P                 # 6272

FP = mybir.dt.float32
BF = mybir.dt.bfloat16
F16 = mybir.dt.float16
F8 = mybir.dt.float8e4
DR = mybir.MatmulPerfMode.DoubleRow

NP_BF = ml_dtypes.bfloat16
NP_F8 = ml_dtypes.float8_e4m3fn

# power-of-2 fp8 scaling (exact): stream = true * SCALE
SQ = 8.0        # q, v (+ residual), k edge rows
SW_ = 128.0     # fp8 weight matrices
SPROJ = SQ * SW_   # common scale of the merged qh/vh/S PSUM region

# Pool g-shares of the two 2048-elem product tensors
PG_PROD = 2
PG_P2 = 1


# ------------------------------------------------------- sync-wait splitting
# The staged walrus accepts only ONE sync-wait command per instruction.
# Tile attaches several waits to some instructions.  Post-pass: hoist all but
# one wait of each over-limit instruction onto same-engine Drain carriers
# placed immediately before it (engine streams execute in block order, so
# "all waits hold before the instruction runs" is preserved).
_WS_COUNTER = [0]


def _split_sync_waits(nc, maxw=1):
    for f in nc.m.functions:
        for blk in f.blocks:
            insts = blk.instructions
            out = []
            changed = False
            for ins in insts:
                si = ins.sync_info
                if si is not None and len(si.on_wait) > maxw:
                    waits = list(si.on_wait)
                    k = len(waits) - maxw
                    for i in range(0, k, maxw):
                        _WS_COUNTER[0] += 1
                        d = mybir.InstDrain(
                            name=f"I-wsplit-{_WS_COUNTER[0]}", ins=[], outs=[]
                        )
                        d.engine = ins.engine
                        d.sync_info = mybir.SyncInfo(
                            on_wait=waits[i : i + maxw], on_update=[]
                        )
                        out.append(d)
                    si.on_wait = waits[k:]
                    changed = True
                out.append(ins)
            if changed:
                blk.instructions = out


# ------------------------------------------------------------- device kernel
def build_nc(pairs_per_block, windows, bias_flags, split_waits=True):
    """Build the SPMD Bass module.

    pairs_per_block[b] = 2-deep edge pair-tiles in block b (same across
    cores).  windows[b] = list of (doff, W) per pair-tile (cross-core
    union).  bias_flags = (has_bq, has_bk, has_bv, has_bc).
    """
    SW = [int(sum(w for _, w in wb)) for wb in windows]   # one-hot cols/block
    has_bq, has_bk, has_bv, has_bc = bias_flags

    nc = bass.Bass("TRN2", target_bir_lowering=False, debug=False,
                   num_devices=NCORES)

    # per-core inputs; qvr/ko/out DMA in 2-block pairs (single shared HWDGE
    # queue at ~625ns per dma_start)
    # qvr j-dim: (q8-lo, q8-hi, v8-lo, v8-hi, r8-lo, r8-hi), all *SQ
    qvr_d = nc.dram_tensor("qvr", [P, NB, 6, P], F8, kind="ExternalInput")
    KOW = [int(pairs_per_block[b]) * 2 * EMB + SW[b] for b in range(NB)]
    ko_d = nc.dram_tensor("ko", [P, sum(KOW)], F8, kind="ExternalInput")
    # fp8 weights [ch%128, ch//128, out] * SW_
    Wq8 = nc.dram_tensor("Wq8", [P, 2, EMB], F8, kind="ExternalInput")
    Wk8 = nc.dram_tensor("Wk8", [P, 2, EMB], F8, kind="ExternalInput")
    Wv8 = nc.dram_tensor("Wv8", [P, 2, EMB], F8, kind="ExternalInput")  # perm
    Rv8 = nc.dram_tensor("Rv8", [P, 2, EMB], F8, kind="ExternalInput")  # perm
    # bf16 Wc^T [(h,d)%128, (h,d)//128, c'] true scale
    Wc16 = nc.dram_tensor("Wc16", [P, 2, EMB], BF, kind="ExternalInput")
    invc_d = nc.dram_tensor("invc", [P, NB], FP, kind="ExternalInput")
    if has_bq or has_bv:
        bqv_d = nc.dram_tensor("bqv", [1, 2 * EMB], BF, kind="ExternalInput")
    if has_bk:
        bk_d = nc.dram_tensor("bk", [1, EMB], BF, kind="ExternalInput")
        cnt_d = nc.dram_tensor("cnt", [1, NPC_PAD], BF, kind="ExternalInput")
    if has_bc:
        bc_d = nc.dram_tensor("bc", [1, EMB], BF, kind="ExternalInput")

    outT_d = nc.dram_tensor("outT", [P, NB, 2, P], F16, kind="ExternalOutput")

    ko_off = [0]
    for b in range(NB):
        ko_off.append(ko_off[-1] + KOW[b])
    KOW2MAX = max(KOW[b] + (KOW[b + 1] if b + 1 < NB else 0)
                  for b in range(0, NB, 2))

    with tile.TileContext(nc) as tc:
        with (
            tc.tile_pool(name="const", bufs=1) as cp,
            tc.tile_pool(name="work", bufs=6) as wp,
            tc.tile_pool(name="qvl", bufs=11) as ql,
            tc.tile_pool(name="kep", bufs=6) as kp,
            tc.tile_pool(name="prd", bufs=4) as pr,
            tc.tile_pool(name="ps_u", bufs=1, space="PSUM") as pu,
            tc.tile_pool(name="ps_qs", bufs=1, space="PSUM") as pqs,
            tc.tile_pool(name="ps_sc", bufs=1, space="PSUM") as psc,
            tc.tile_pool(name="ps_ov", bufs=2, space="PSUM") as pov,
            tc.tile_pool(name="ps_o", bufs=1, space="PSUM") as po,
        ):
            # ---------------- constants
            ident = cp.tile([P, P], BF)
            make_identity(nc, ident[:])
            ident82 = cp.tile([P, 2, P], F8)     # identity pair for DR d-red
            nc.scalar.copy(ident82[:, 0, :], ident[:])
            nc.scalar.copy(ident82[:, 1, :], ident[:])
            zf82 = cp.tile([P, 2, P], F8)
            nc.vector.memset(zf82[:], 0.0)
            if has_bq or has_bv or has_bk or has_bc:
                ones1 = cp.tile([1, P], BF)
                nc.vector.memset(ones1[:], 1.0)

            wt = {}
            _const_dmas = []
            for nm, t, dt_ in (("Wq", Wq8, F8), ("Wk", Wk8, F8),
                               ("Wv", Wv8, F8), ("Rv", Rv8, F8),
                               ("Wc", Wc16, BF)):
                s = cp.tile([P, 2, EMB], dt_, tag=f"w{nm}")
                _const_dmas.append((s, t))
                wt[nm] = s
            invc_sb = cp.tile([P, NB], FP)
            _const_dmas.append((invc_sb, invc_d))
            if has_bq or has_bv:
                bqv_sb = cp.tile([1, 2 * EMB], BF, tag="bqv")
                nc.sync.dma_start(bqv_sb[:], bqv_d[:])
            if has_bk:
                bk_sb = cp.tile([1, EMB], BF, tag="bk")
                nc.sync.dma_start(bk_sb[:], bk_d[:])
                cnt_sb = cp.tile([1, NPC_PAD], BF)
                nc.sync.dma_start(cnt_sb[:], cnt_d[:])
            if has_bc:
                bc_sb = cp.tile([1, EMB], BF, tag="bc")
                nc.sync.dma_start(bc_sb[:], bc_d[:])

            st = {}

            # ---------------- stages (software pipeline over block PAIRS)
            # Post-S1 stages process a pair of blocks per op: ACT/DVE/Pool
            # per-op init costs are paid once per pair, and PSUM pair-tiles
            # use first-writer-zeroes (start=True only on each bank's first
            # matmul group; later groups accumulate onto the pending-zero
            # bytes, the same HW-proven idiom as the S1 window resets).
            def S0(m):      # SP: fetch pair m (blocks 2m, 2m+1)
                b = 2 * m
                qvr = ql.tile([P, 2, 6, P], F8, tag="qvr")
                hi = min(b + 2, NB)
                nc.sync.dma_start(qvr[:, 0:hi - b, :, :], qvr_d[:, b:hi, :, :])
                ko = kp.tile([P, KOW2MAX], F8, tag="ko")
                import os as _o
                if _o.environ.get("KSPLIT", "0") == "1":
                    mid = ko_off[min(b + 1, hi)]
                    nc.sync.dma_start(ko[:, 0:mid - ko_off[b]],
                                      ko_d[:, ko_off[b]:mid])
                    if mid < ko_off[hi]:
                        nc.sync.dma_start(
                            ko[:, mid - ko_off[b]:ko_off[hi] - ko_off[b]],
                            ko_d[:, mid:ko_off[hi]])
                else:
                    w = ko_off[hi] - ko_off[b]
                    nc.sync.dma_start(ko[:, 0:w],
                                      ko_d[:, ko_off[b]:ko_off[hi]])
                st["qvr", m] = qvr
                st["ko", m] = ko

            def blocks_of(m):
                b0 = 2 * m
                return [b0] if b0 + 1 >= NB else [b0, b0 + 1]

            def S1(m):      # PE: U^T pair accumulation (DR, shared one-hot)
                ko = st.pop(("ko", m))
                blks = blocks_of(m)
                ps_uT = pu.tile([P, 2, 2, P], FP, space="PSUM", tag="uT")
                # full-width zero reset (write-mode), then accumulate
                mms = []
                for j in range(2):
                    for hf in range(2):
                        mms.append(dict(out=ps_uT[:, j, hf, :], lhsT=zf82[:],
                                        rhs=zf82[:], start=True, stop=False,
                                        perf_mode=DR, skip_group_check=True))
                for b in blks:
                    base = ko_off[b] - ko_off[2 * m]
                    T = int(pairs_per_block[b])
                    ke = ko[:, base:base + T * 2 * EMB].rearrange(
                        "p (t two c) -> p t two c", two=2, c=EMB)
                    oh = ko[:, base + T * 2 * EMB:base + T * 2 * EMB + SW[b]]
                    wo = 0
                    for t in range(T):
                        doff, W = windows[b][t]
                        if W > 0:
                            ohb = oh[:, wo:wo + W].unsqueeze(1).to_broadcast(
                                [P, 2, W])
                            for hf in range(2):
                                mms.append(dict(
                                    out=ps_uT[:, b % 2, hf, doff:doff + W],
                                    lhsT=ke[:, t, :, hf * P:(hf + 1) * P],
                                    rhs=ohb, start=False, stop=False,
                                    perf_mode=DR, skip_group_check=True))
                        wo += W
                    mms[-1]["stop"] = b == blks[-1]
                for kw in mms:
                    nc.tensor.matmul(**kw)
                st["ps_uT", m] = ps_uT

            def S1c(m):     # ACT op1: U^T pair -> SBUF fp8 (stays *SQ)
                ps_uT = st.pop(("ps_uT", m))
                uT_sb = wp.tile([P, 2, 2, P], F8, tag="uT_sb")
                nc.scalar.copy(uT_sb[:], ps_uT[:])
                st["uT_sb", m] = uT_sb

            def S2(m):      # PE: q/v/S projections, pair -> one x1024 region
                qvr = st.pop(("qvr", m))
                uT_sb = st.pop(("uT_sb", m))
                ps_qs = pqs.tile([P, 2, 3 * EMB], FP, space="PSUM", tag="qs")
                for j, b in enumerate(blocks_of(m)):
                    reg = ps_qs[:, j, :]
                    nc.tensor.matmul(out=reg[:, 0:EMB],
                                     lhsT=qvr[:, j, 0:2, :], rhs=wt["Wq"][:],
                                     start=True, stop=not has_bq,
                                     perf_mode=DR, skip_group_check=True)
                    if has_bq:
                        nc.tensor.matmul(out=reg[:, 0:EMB], lhsT=ones1[:],
                                         rhs=bqv_sb[:, 0:EMB],
                                         start=False, stop=True,
                                         skip_group_check=True)
                    nc.tensor.matmul(out=reg[:, EMB:2 * EMB],
                                     lhsT=qvr[:, j, 2:4, :], rhs=wt["Wv"][:],
                                     start=True, stop=False,
                                     perf_mode=DR, skip_group_check=True)
                    nc.tensor.matmul(out=reg[:, EMB:2 * EMB],
                                     lhsT=qvr[:, j, 2:4, :], rhs=wt["Rv"][:],
                                     start=False, stop=False,
                                     perf_mode=DR, skip_group_check=True)
                    nc.tensor.matmul(out=reg[:, EMB:2 * EMB],
                                     lhsT=qvr[:, j, 4:6, :], rhs=wt["Wv"][:],
                                     start=False, stop=not has_bv,
                                     perf_mode=DR, skip_group_check=True)
                    if has_bv:
                        nc.tensor.matmul(out=reg[:, EMB:2 * EMB],
                                         lhsT=ones1[:],
                                         rhs=bqv_sb[:, EMB:2 * EMB],
                                         start=False, stop=True,
                                         skip_group_check=True)
                    nc.tensor.matmul(out=reg[:, 2 * EMB:3 * EMB],
                                     lhsT=uT_sb[:, j, :, :], rhs=wt["Wk"][:],
                                     start=True, stop=not has_bk,
                                     perf_mode=DR, skip_group_check=True)
                    if has_bk:
                        nc.tensor.matmul(
                            out=reg[:, 2 * EMB:3 * EMB],
                            lhsT=cnt_sb[:, b * P:(b + 1) * P],
                            rhs=bk_sb[:], start=False, stop=True,
                            skip_group_check=True)
                st["ps_qs", m] = ps_qs

            def S2c(m):     # ACT op2: merged qh/vh/S pair copy
                ps_qs = st.pop(("ps_qs", m))
                qs_sb = ql.tile([P, 2, 3 * EMB], BF, tag="qs_sb")
                nc.scalar.mul(qs_sb[:], ps_qs[:], 1.0 / SPROJ)
                st["qs_sb", m] = qs_sb

            def S3b(m):     # DVE: prod bf16; Pool: fp8 g-tail (per half
                            # -- ISA free-dim patterns are 3D max)
                qs_sb = st[("qs_sb", m)]
                gs = H - PG_PROD
                prod = pr.tile([P, 2, H, gs, D], BF, tag="prod")
                prod8 = pr.tile([P, 2, H, PG_PROD, D], F8, tag="prod8")
                for j in range(2):
                    qh = qs_sb[:, j, 0:EMB].rearrange(
                        "p (h d) -> p h d", h=H)
                    sg = qs_sb[:, j, 2 * EMB:3 * EMB].rearrange(
                        "p (g d) -> p g d", g=H)
                    nc.vector.tensor_tensor(
                        out=prod[:, j],
                        in0=qh.unsqueeze(2).to_broadcast([P, H, gs, D]),
                        in1=sg[:, 0:gs, :].unsqueeze(1).to_broadcast(
                            [P, H, gs, D]),
                        op=mybir.AluOpType.mult)
                    nc.gpsimd.tensor_tensor(
                        out=prod8[:, j],
                        in0=qh.unsqueeze(2).to_broadcast(
                            [P, H, PG_PROD, D]),
                        in1=sg[:, gs:H, :].unsqueeze(1).to_broadcast(
                            [P, H, PG_PROD, D]),
                        op=mybir.AluOpType.mult)
                st["prod", m] = prod
                st["prod8", m] = prod8

            def S3p(m):     # PE: pair d-red (32 bf16 + 2x16 fp8-DR)
                prod = st.pop(("prod", m))
                prod8 = st.pop(("prod8", m))
                gs = H - PG_PROD
                ps_sc = psc.tile([P, 2, H, H], FP, space="PSUM", tag="sc")
                for dd in range(D):
                    nc.tensor.matmul(out=ps_sc[:, :, :, 0:gs], lhsT=ident[:],
                                     rhs=prod[:, :, :, :, dd],
                                     start=(dd == 0), stop=False,
                                     skip_group_check=True)
                for j in range(2):
                    for jj in range(D // 2):
                        nc.tensor.matmul(
                            out=ps_sc[:, j, :, gs:H],
                            lhsT=ident82[:],
                            rhs=prod8[:, j, :, :, 2 * jj:2 * jj + 2
                                      ].rearrange("p h g d -> p d (h g)"),
                            start=(jj == 0),
                            stop=(jj == D // 2 - 1) and j == 1,
                            perf_mode=DR, skip_group_check=True)
                st["ps_sc", m] = ps_sc

            def S3f(m):     # ACT (last): per-block exp from PSUM
                ps_sc = st.pop(("ps_sc", m))
                ex = wp.tile([P, 2, H, H], BF, tag="ex")
                for j, b in enumerate(blocks_of(m)):
                    nc.scalar.activation(
                        out=ex[:, j, :, :], in_=ps_sc[:, j, :, :],
                        func=mybir.ActivationFunctionType.Exp,
                        scale=invc_sb[:, b:b + 1])
                st["ex", m] = ex

            def S4r(m):     # DVE: pair den + rden; Pool: att = ex * rden
                ex = st.pop(("ex", m))
                den = wp.tile([P, 2, H], FP, tag="den")
                nc.vector.tensor_reduce(den[:], ex[:],
                                        axis=mybir.AxisListType.X,
                                        op=mybir.AluOpType.add)
                rden = wp.tile([P, 2, H], BF, tag="rden")
                with nc.allow_low_precision(
                        reason="bf16 softmax normalization, 2^-8 rel err"):
                    nc.vector.reciprocal(rden[:], den[:])
                att = wp.tile([P, 2, H, H], BF, tag="att")
                nc.gpsimd.tensor_tensor(
                    out=att[:], in0=ex[:],
                    in1=rden[:].unsqueeze(3).to_broadcast([P, 2, H, H]),
                    op=mybir.AluOpType.mult)
                st["att", m] = att

            def S4b(m):     # DVE+Pool: p2 = att (x) vh (per half)
                qs_sb = st.pop(("qs_sb", m))
                att = st.pop(("att", m))
                p2 = pr.tile([P, 2, H, D, H], BF, tag="p2")
                gs = H - PG_P2
                for j in range(2):
                    vh = qs_sb[:, j, EMB:2 * EMB].rearrange(
                        "p (d g) -> p d g", d=D)
                    nc.vector.tensor_tensor(
                        out=p2[:, j, :, :, 0:gs],
                        in0=att[:, j, :, 0:gs].unsqueeze(2).to_broadcast(
                            [P, H, D, gs]),
                        in1=vh[:, :, 0:gs].unsqueeze(1).to_broadcast(
                            [P, H, D, gs]),
                        op=mybir.AluOpType.mult)
                    nc.gpsimd.tensor_tensor(
                        out=p2[:, j, :, :, gs:H],
                        in0=att[:, j, :, gs:H].unsqueeze(2).to_broadcast(
                            [P, H, D, PG_P2]),
                        in1=vh[:, :, gs:H].unsqueeze(1).to_broadcast(
                            [P, H, D, PG_P2]),
                        op=mybir.AluOpType.mult)
                st["p2", m] = p2

            def S5a(m):     # PE: pair transposing g-sum -> ov^T in PSUM
                p2 = st.pop(("p2", m))
                ps_ov = pov.tile([P, 2, 2, P], FP, space="PSUM", tag="ov")
                for j, b in enumerate(blocks_of(m)):
                    for hf in range(2):
                        for g in range(H):
                            lhsT = p2[:, j, 4 * hf:4 * hf + 4, :, g
                                      ].rearrange("p h d -> p (h d)")
                            nc.tensor.matmul(
                                out=ps_ov[:, j, hf, :], lhsT=lhsT,
                                rhs=ident[:],
                                start=(g == 0), stop=(g == H - 1),
                                skip_group_check=True)
                st["ps_ov", m] = ps_ov

            def S5b(m):     # ACT op3: ov^T pair -> SBUF bf16
                ps_ov = st.pop(("ps_ov", m))
                ovT = wp.tile([P, 2, 2, P], BF, tag="ovT")
                nc.scalar.copy(ovT[:], ps_ov[:])
                st["ovT", m] = ovT

            def S5c(m):     # PE: pair out^T = Wc^T @ ov^T (bf16)
                ovT = st.pop(("ovT", m))
                ps_o = po.tile([P, 2, 2, P], FP, space="PSUM", tag="o")
                for j, b in enumerate(blocks_of(m)):
                    for cf in range(2):
                        for hh in range(2):
                            nc.tensor.matmul(
                                out=ps_o[:, j, cf, :],
                                lhsT=wt["Wc"][:, hh, cf * P:(cf + 1) * P],
                                rhs=ovT[:, j, hh, :],
                                start=(hh == 0),
                                stop=(hh == 1) and not has_bc,
                                skip_group_check=True)
                        if has_bc:
                            nc.tensor.matmul(
                                out=ps_o[:, j, cf, :],
                                lhsT=bc_sb[:, cf * P:(cf + 1) * P],
                                rhs=ones1[:],
                                start=False, stop=True,
                                skip_group_check=True)
                st["ps_o", m] = ps_o

            def S5d(m):     # ACT op4: fp16 pair out; SP: DMA pair
                ps_o = st.pop(("ps_o", m))
                fo = ql.tile([P, 2, 2, P], F16, tag="fo")
                nc.scalar.copy(fo[:], ps_o[:])
                b = 2 * m
                hi = min(b + 2, NB)
                nc.sync.dma_start(outT_d[:, b:hi, :, :], fo[:, 0:hi - b, :, :])

            # list order = per-engine priority order; exp (S3f) last so its
            # wait on the same-iteration d-reduce doesn't delay the copies
            import os
            _p = os.environ.get("KPERM", "7")
            if _p == "0":
                stages = [(0, S0), (3, S2), (1, S1), (2, S1c), (3, S2c),
                          (4, S3b), (5, S3p), (6, S4r), (7, S4b), (8, S5a),
                          (9, S5b), (9, S5c), (10, S5d), (5, S3f)]
            elif _p == "1":   # copies earlier in priority
                stages = [(0, S0), (2, S1c), (3, S2c), (9, S5b), (3, S2),
                          (1, S1), (4, S3b), (5, S3p), (6, S4r), (7, S4b),
                          (8, S5a), (9, S5c), (10, S5d), (5, S3f)]
            elif _p == "2":   # tail stages high priority
                stages = [(0, S0), (9, S5b), (9, S5c), (10, S5d), (8, S5a),
                          (7, S4b), (6, S4r), (5, S3p), (4, S3b), (3, S2),
                          (3, S2c), (2, S1c), (1, S1), (5, S3f)]
            elif _p == "4":   # tail-first but copies before compute
                stages = [(0, S0), (10, S5d), (9, S5b), (2, S1c), (3, S2c),
                          (9, S5c), (8, S5a), (7, S4b), (6, S4r), (5, S3p),
                          (4, S3b), (3, S2), (1, S1), (5, S3f)]
            elif _p == "5":   # S0 after tail stages
                stages = [(10, S5d), (9, S5b), (9, S5c), (0, S0), (8, S5a),
                          (7, S4b), (6, S4r), (5, S3p), (4, S3b), (3, S2),
                          (3, S2c), (2, S1c), (1, S1), (5, S3f)]
            elif _p == "7":
                stages = [(0, S0), (10, S5d), (9, S5b), (9, S5c), (8, S5a),
                          (7, S4b), (6, S4r), (5, S3p), (5, S3f), (4, S3b),
                          (3, S2), (3, S2c), (2, S1c), (1, S1)]
            elif _p == "8":
                stages = [(0, S0), (5, S3p), (5, S3f), (9, S5b), (9, S5c),
                          (10, S5d), (8, S5a), (7, S4b), (6, S4r), (4, S3b),
                          (3, S2), (3, S2c), (2, S1c), (1, S1)]
            elif _p == "3":   # DVE/Pool work first
                stages = [(0, S0), (4, S3b), (7, S4b), (6, S4r), (3, S2),
                          (1, S1), (2, S1c), (3, S2c), (5, S3p), (8, S5a),
                          (9, S5b), (9, S5c), (10, S5d), (5, S3f)]
            DEPTH = 11
            import os as _os
            _pf = _os.environ.get("KPF", "1") == "1"
            NP2 = (NB + 1) // 2
            for i in range(NP2 + DEPTH - 1):
                if i == 1:
                    # consts issue after the first input prefetches so the
                    # first ko stream heads the DMA queue (needed @ iter 3)
                    for s_, t_ in _const_dmas:
                        nc.sync.dma_start(s_[:], t_[:])
                for off, fn in stages:
                    if fn is S0 and _pf:
                        if i == 0:
                            fn(0)
                            fn(1)
                            fn(2)
                        if i + 3 < NP2:
                            fn(i + 3)
                        continue
                    mm = i - off
                    if 0 <= mm < NP2:
                        fn(mm)

    if split_waits:
        _split_sync_waits(nc)
    return nc


# --------------------------------------------------------------- host prep
def _prep(q, k, v, edge_index, Wq, bq, Wk, bk, Wv, bv, Wc, bc):
    A = np.asarray(edge_index[0], dtype=np.int64)
    B = np.asarray(edge_index[1], dtype=np.int64)
    order = np.argsort(A, kind="stable")
    A_s = A[order]
    B_s = B[order]

    core_lo = np.searchsorted(A_s, np.arange(NCORES) * NPC, side="left")
    core_hi = np.searchsorted(A_s, (np.arange(NCORES) + 1) * NPC, side="left")

    # --- per-core 2-deep slot assignment (vectorized)
    per_core = []
    npair = np.zeros((NCORES, NB), dtype=np.int64)
    for o in range(NCORES):
        a = A_s[core_lo[o]:core_hi[o]] - o * NPC      # local dest, ascending
        bi = B_s[core_lo[o]:core_hi[o]]
        n = len(a)
        first = np.searchsorted(a, a, side="left")
        rank = np.arange(n) - first
        depth = rank % 2
        cnt_d = np.bincount(a, minlength=NPC_PAD)
        s_d = (cnt_d + 1) // 2
        s_cum = np.cumsum(s_d) - s_d                   # global slot prefix
        blk_start = s_cum[np.arange(NB) * P]           # first slot of block
        slot_in_blk = (s_cum[a] - blk_start[a // P]) + rank // 2
        t = slot_in_blk // P
        p = slot_in_blk % P
        blk = a // P
        np.maximum.at(npair[o], blk, t + 1)
        per_core.append((a, bi, blk, t, p, depth))
    pairs_per_block = np.maximum(1, npair.max(axis=0)).astype(int)

    # --- per-(block,tile) destination windows, union across cores
    TMAX = int(pairs_per_block.max())
    lo = np.full((NB, TMAX), P, dtype=np.int64)
    hi = np.full((NB, TMAX), -1, dtype=np.int64)
    for o in range(NCORES):
        a, bi, blk, t, p, depth = per_core[o]
        dl = a - blk * P
        np.minimum.at(lo, (blk, t), dl)
        np.maximum.at(hi, (blk, t), dl)
    windows = []
    for bidx in range(NB):
        wb = []
        for t in range(int(pairs_per_block[bidx])):
            if hi[bidx, t] < 0:
                wb.append((0, 0))
                continue
            doff = int(lo[bidx, t])
            W = int(hi[bidx, t]) - doff + 1
            W = min((W + 3) // 4 * 4, P - doff)
            wb.append((doff, W))
        windows.append(wb)
    SW = [sum(w for _, w in wb) for wb in windows]

    KOW = [int(pairs_per_block[b]) * 2 * EMB + SW[b] for b in range(NB)]
    ko_off = np.zeros(NB + 1, dtype=np.int64)
    ko_off[1:] = np.cumsum(KOW)
    ke_base = ko_off[:NB]
    oh_base = ko_off[:NB] + pairs_per_block * 2 * EMB
    oh_col = np.zeros((NB, TMAX), dtype=np.int64)
    doffs = np.zeros((NB, TMAX), dtype=np.int64)
    for bidx in range(NB):
        acc = 0
        for t in range(int(pairs_per_block[bidx])):
            oh_col[bidx, t] = acc
            acc += windows[bidx][t][1]
            doffs[bidx, t] = windows[bidx][t][0]

    k8 = (np.asarray(k, np.float32) * SQ).astype(NP_F8)
    kos = []
    for o in range(NCORES):
        a, bi, blk, t, p, depth = per_core[o]
        ko = np.zeros((P, int(ko_off[-1])), dtype=NP_F8)
        cstart = ke_base[blk] + t * 2 * EMB + depth * EMB
        cidx = cstart[:, None] + np.arange(EMB)[None, :]
        ko[p[:, None], cidx] = k8[bi]
        m0 = depth == 0
        cols = (oh_base[blk[m0]] + oh_col[blk[m0], t[m0]]
                + (a[m0] - blk[m0] * P - doffs[blk[m0], t[m0]]))
        ko[p[m0], cols] = 1.0
        kos.append(ko)

    cnt_nodes = np.bincount(A, minlength=N_NODES).astype(np.float32)
    invc_full = 1.0 / np.maximum(cnt_nodes, 1.0)
    invcs, cnts = [], []
    for o in range(NCORES):
        s = np.ones(NPC_PAD, dtype=np.float32)
        s[:NPC] = invc_full[o * NPC:(o + 1) * NPC]
        invcs.append(np.ascontiguousarray(s.reshape(NB, P).T))
        c = np.zeros(NPC_PAD, dtype=np.float32)
        c[:NPC] = cnt_nodes[o * NPC:(o + 1) * NPC]
        cnts.append((c * SPROJ).reshape(1, NPC_PAD).astype(NP_BF))

    # q fp8; v as fp8 value + fp8 residual (both *SQ, exact power-of-2)
    q8 = (np.asarray(q, np.float32) * SQ).astype(NP_F8)
    vs = np.asarray(v, np.float32) * SQ
    v8 = vs.astype(NP_F8)
    r8 = (vs - v8.astype(np.float32)).astype(NP_F8)
    qvrs = []
    for o in range(NCORES):
        qvr = np.zeros((P, NB, 6, P), dtype=NP_F8)
        for j, src in ((0, q8), (2, v8), (4, r8)):
            sT = np.zeros((EMB, NPC_PAD), dtype=NP_F8)
            sT[:, :NPC] = src[o * NPC:(o + 1) * NPC].T
            qvr[:, :, j, :] = sT[0:P].reshape(P, NB, P)
            qvr[:, :, j + 1, :] = sT[P:EMB].reshape(P, NB, P)
        qvrs.append(qvr)

    # Wv column permutation: vh lands as [n, (d, g)]
    WvT = np.ascontiguousarray(np.asarray(Wv, np.float32).T)
    WvT_perm = WvT.reshape(EMB, H, D).transpose(0, 2, 1).reshape(EMB, EMB)
    bv_perm = np.asarray(bv, np.float32).reshape(H, D).T.reshape(-1)

    def pack2(WT):
        # [ch, out] -> [ch%128, ch//128, out]
        a = np.ascontiguousarray(np.asarray(WT, np.float32)).reshape(
            2, P, EMB)
        return np.ascontiguousarray(a.transpose(1, 0, 2))

    Wv_s = WvT_perm * SW_
    Wv8 = Wv_s.astype(NP_F8)
    Rv8 = (Wv_s - Wv8.astype(np.float32)).astype(NP_F8)

    bias_flags = (bool(np.any(np.asarray(bq))), bool(np.any(np.asarray(bk))),
                  bool(np.any(np.asarray(bv))), bool(np.any(np.asarray(bc))))
    has_bq, has_bk, has_bv, has_bc = bias_flags

    com = {
        "Wq8": pack2(np.asarray(Wq, np.float32).T * SW_).astype(NP_F8),
        "Wk8": pack2(np.asarray(Wk, np.float32).T * SW_).astype(NP_F8),
        "Wv8": pack2(Wv8.astype(np.float32)).astype(NP_F8),
        "Rv8": pack2(Rv8.astype(np.float32)).astype(NP_F8),
        "Wc16": pack2(np.asarray(Wc, np.float32).T).astype(NP_BF),
    }
    if has_bq or has_bv:
        bqv = np.concatenate([np.asarray(bq, np.float32),
                              bv_perm]) * SPROJ
        com["bqv"] = bqv.reshape(1, 2 * EMB).astype(NP_BF)
    if has_bk:
        com["bk"] = np.asarray(bk, np.float32).reshape(1, EMB).astype(NP_BF)
    if has_bc:
        com["bc"] = np.asarray(bc, np.float32).reshape(1, EMB).astype(NP_BF)

    in_maps = []
    for o in range(NCORES):
        m = dict(com)
        m["qvr"] = qvrs[o]
        m["ko"] = kos[o]
        m["invc"] = invcs[o]
        if has_bk:
            m["cnt"] = cnts[o]
        in_maps.append(m)
    return pairs_per_block.tolist(), windows, bias_flags, in_maps


_LAST = {}


def kernel(q, k, v, edge_index, Wq, bq, Wk, bk, Wv, bv, Wc, bc, latent=None,
           _want_results=False, _trace=False):
    pairs_per_block, windows, bias_flags, in_maps = _prep(
        q, k, v, edge_index, Wq, bq, Wk, bk, Wv, bv, Wc, bc)
    key = str((pairs_per_block, windows, bias_flags))
    if _LAST.get("key") != key:
        _LAST["nc"] = build_nc(pairs_per_block, windows, bias_flags)
        _LAST["key"] = key
    nc = _LAST["nc"]

    res = run_bass_kernel_spmd(nc, in_maps, core_ids=list(range(NCORES)),
                               trace=_trace)
    out = np.empty((N_NODES, EMB), dtype=np.float32)
    for o in range(NCORES):
        oT = res.results[o]["outT"].astype(np.float32)   # [P, NB, 2, P]
        full = np.empty((EMB, NPC_PAD), dtype=np.float32)
        full[0:P] = oT[:, :, 0, :].reshape(P, NPC_PAD)
        full[P:EMB] = oT[:, :, 1, :].reshape(P, NPC_PAD)
        out[o * NPC:(o + 1) * NPC] = full[:, :NPC].T
    if _want_results:
        return out, res
    return out
